# revision 38
# baseline (speedup 1.0000x reference)
"""GAT (2-layer: 4 heads -> 1 head) on 8 trn2 NeuronCores.

Strategy (dst-partitioned slot design):
 - Relabel nodes by in-degree (ascending) into core-major blocks:
   final id = c*(T*128) + t*128 + d, where sorted 128-node block s = 8t + c.
   Each core owns T = 99 blocks (98 data + 1 all-dummy tail block whose slot
   12544 holds the table2 "dummy row" at the same local address on every
   core). Slab structure K_t is shared by all cores -> one SPMD program.
 - Per super-tile, edges live in "slabs": slab k holds edge k of every dst
   (one edge per partition). Self-loops pinned to slab 0, so the slab-0
   gather also delivers each dst's own table row (alpha_dst for free).
 - Phase 1 (replicated): table1[n] = x[n] @ [W1 | W1@A1s | W1@A1d] (136 f32).
 - Phase 2: per slab, [P,1] indirect-gather of table1[src]; per-partition
   softmax accumulation (exp without max-subtraction -- padded slots gather
   a dummy row with alpha_src=-1e4, so w = exp(leaky(-1e4+a_d)) == 0).
   Epilogue computes h1 then table2_local[t*128+d] = [h2 | a_s2 | a_d2].
 - AllGather table2_local (12672x4) -> table2_full (101376x4, Shared).
 - Phase 3: same slab loop over 16-byte table2 rows -> out2 [12672, 2].
"""
import os
import numpy as np

N = 100000
E = 1600000
IN_DIM = 256
HID = 128
HEADS = 4
C1 = HID // HEADS
OUT = 2
NEG = 0.2
P = 128
NCORES = 8
N_ST_DATA = 784                   # data blocks (ceil(100000/128) = 782 -> 784 for /8)
T_PER_CORE = N_ST_DATA // NCORES + 1   # 99 (incl. all-dummy tail block)
N_ST = T_PER_CORE * NCORES        # 792
N_PAD = N_ST * P                  # 101376
ROWS_PER_CORE = T_PER_CORE * P    # 12672
DUMMY1 = N_PAD                    # table1 dummy row (extra row)
DUMMY2_LOCAL = (T_PER_CORE - 1) * P   # local row 12544 in every core's block
PAD_ALPHA = -1.0e4
D1 = HID + 2 * HEADS              # 136


def _host_prep(x, edge_index, W1, att_src1, att_dst1, b1, W2, att_src2, att_dst2, b2):
    src0 = np.asarray(edge_index[0], dtype=np.int64)
    dst0 = np.asarray(edge_index[1], dtype=np.int64)
    loops = np.arange(N, dtype=np.int64)
    src = np.concatenate([src0, loops])
    dst = np.concatenate([dst0, loops])

    deg = np.bincount(dst, minlength=N)
    order = np.argsort(deg, kind="stable")        # sorted position -> old id

    # sorted position p = s*128+d, block s = 8t+c  ->  fid = (c*T + t)*128 + d
    n_data = N_ST_DATA * P                        # 100352 sorted slots
    p_ar = np.arange(n_data, dtype=np.int64)
    s_ar = p_ar // P
    d_ar = p_ar % P
    t_ar = s_ar // NCORES
    c_ar = s_ar % NCORES
    fid_of_sorted = (c_ar * T_PER_CORE + t_ar) * P + d_ar   # [n_data]

    fid_of_old = np.empty(N, dtype=np.int64)
    fid_of_old[order] = fid_of_sorted[:N]
    old_of_fid = np.full(N_PAD, -1, dtype=np.int64)
    old_of_fid[fid_of_old] = np.arange(N)

    nsrc = fid_of_old[src]
    ndst = fid_of_old[dst]
    ndeg = np.zeros(N_PAD, dtype=np.int64)
    ndeg[fid_of_old] = deg

    K_ct = ndeg.reshape(NCORES, T_PER_CORE, P).max(-1)      # [8, 99]
    K_t = np.maximum(K_ct.max(0), 1)                        # [99]
    tot_slabs = int(K_t.sum())
    slab_off = np.concatenate([[0], np.cumsum(K_t)]).astype(np.int64)

    # slot index k of each edge within its dst; self-loop forced to k=0
    notself = (nsrc != ndst).astype(np.int8)
    key = np.lexsort((notself, ndst))
    ds_ = ndst[key]
    ss_ = nsrc[key]
    counts = np.bincount(ds_, minlength=N_PAD)
    run_start = np.zeros(N_PAD + 1, dtype=np.int64)
    run_start[1:] = np.cumsum(counts)
    kk = np.arange(ds_.size, dtype=np.int64) - run_start[ds_]

    blk = ds_ // P
    core = blk // T_PER_CORE
    t_idx = blk % T_PER_CORE
    d_rel = ds_ % P
    col = slab_off[t_idx] + kk

    idx1 = np.full((NCORES, P, tot_slabs), DUMMY1, dtype=np.int32)
    idx1[core, d_rel, col] = ss_.astype(np.int32)
    idx2 = idx1.copy()
    for c in range(NCORES):
        pad_val = c * ROWS_PER_CORE + DUMMY2_LOCAL
        m = idx2[c] == DUMMY1
        idx2[c][m] = pad_val

    # ---- phase-3 windowed dma_gather structures (currently unused) ----
    # table2big rows (256B bf16) indexed by fid; 4 windows of WS rows with
    # int16 window-local indices.  Self-loops handled separately (slab-0
    # style indirect gather on the compact table2), so windows hold only
    # non-self edges.  Window dummy = each window's resident core-tail
    # dummy row (alpha_src = -1e4 -> weight 0).
    WS = 32768
    NW = 4
    wdum = np.array([12544, 37888, 75904, 101248], dtype=np.int64)
    assert all(wdum // WS == np.arange(NW))
    ns_mask = ss_ != ds_
    dsn = ds_[ns_mask]
    ssn = ss_[ns_mask]
    wn = np.minimum(ssn // WS, NW - 1)
    blk_n = dsn // P
    core_n = blk_n // T_PER_CORE
    t_n = blk_n % T_PER_CORE
    p_n = dsn % P
    # per (core, t, p, w) counts and slot index within that bucket
    keyw = np.lexsort((ssn, wn, dsn))
    dsw = dsn[keyw]; ssw = ssn[keyw]; www = wn[keyw]
    bucket = dsw * NW + www
    cntw = np.bincount(bucket, minlength=N_PAD * NW)
    rs = np.zeros(N_PAD * NW + 1, dtype=np.int64)
    rs[1:] = np.cumsum(cntw)
    kw = np.arange(dsw.size, dtype=np.int64) - rs[bucket]
    cntw4 = cntw.reshape(N_PAD, NW)
    # K3_tw[t, w] = max over cores and dsts of row t (shared SPMD shape)
    K3_tw = cntw4.reshape(NCORES, T_PER_CORE, P, NW).max(2).max(0)  # [T, W]
    K3_tw = K3_tw[:T_PER_CORE - 1]          # data tiles only
    K3_t = K3_tw.sum(1)                      # [98]
    w_off = np.zeros((T_PER_CORE - 1, NW), dtype=np.int64)
    w_off[:, 1:] = np.cumsum(K3_tw, axis=1)[:, :-1]
    # flat int16 idx grid per core: for (t, w): [K3_tw, 128] slots
    # (k-major), value = window-local src fid (or window dummy)
    cols_t = (K3_tw * P // 16).sum(1)        # int16 cols per tile
    colbase = np.zeros(T_PER_CORE, dtype=np.int64)
    colbase[1:] = np.cumsum(cols_t)
    totcols = int(colbase[T_PER_CORE - 1])
    idx3 = np.empty((NCORES, 16, totcols), dtype=np.int16)
    for c in range(NCORES):
        # per-(t, w) slot grid, default = window-local dummy
        grid = {(t, w): np.full((int(K3_tw[t, w]), P), wdum[w] - w * WS,
                                dtype=np.int64)
                for t in range(T_PER_CORE - 1) for w in range(NW)}
        m_c = core_n[keyw] == c
        tt = t_n[keyw][m_c]; pp = p_n[keyw][m_c]
        wwc = www[m_c]; kkc = kw[m_c]; ssc = ssw[m_c]
        for t in range(T_PER_CORE - 1):
            mt = tt == t
            for w in range(NW):
                mw = mt & (wwc == w)
                g = grid[(t, w)]
                g[kkc[mw], pp[mw]] = ssc[mw] - w * WS
        pieces = []
        for t in range(T_PER_CORE - 1):
            for w in range(NW):
                g = grid[(t, w)]                     # [Kw, 128] k-major
                if g.size == 0:
                    continue
                fl = g.reshape(-1)                    # flat i = k*128+p
                pieces.append(fl.reshape(-1, 16).T)   # [16, n/16]
        idx3[c] = np.concatenate(pieces, axis=1).astype(np.int16)
    # self-loop (slab-0) indices into compact table2 per (core, tile)
    ixself = np.empty((NCORES, P, T_PER_CORE - 1), dtype=np.int32)
    for c in range(NCORES):
        for t in range(T_PER_CORE - 1):
            base = (c * T_PER_CORE + t) * P
            fids = np.arange(base, base + P)
            real = old_of_fid[fids] >= 0
            v = np.where(real, fids, c * ROWS_PER_CORE + DUMMY2_LOCAL)
            ixself[c, :, t] = v.astype(np.int32)

    # x tiles [792, 128, 2, 128] bf16: [s, p, i, n] = xp[s*128+n, i*128+p]
    # (partition-major contiguous so each block load is one flat 64KB DMA)
    import ml_dtypes
    xp = np.zeros((N_PAD, IN_DIM), dtype=np.float32)
    xp[fid_of_old] = np.asarray(x, dtype=np.float32)
    x_tiles = np.ascontiguousarray(
        xp.reshape(N_ST, P, 2, P).transpose(0, 3, 2, 1)).astype(ml_dtypes.bfloat16)

    A1s = np.zeros((HID, HEADS), dtype=np.float32)
    A1d = np.zeros((HID, HEADS), dtype=np.float32)
    for h in range(HEADS):
        A1s[h * C1:(h + 1) * C1, h] = np.asarray(att_src1, np.float32)[h]
        A1d[h * C1:(h + 1) * C1, h] = np.asarray(att_dst1, np.float32)[h]
    W1_ = np.asarray(W1, np.float32)
    W1aug = np.concatenate([W1_, W1_ @ A1s, W1_ @ A1d], axis=1)       # [256,136]
    W1aug_t = np.ascontiguousarray(
        W1aug.reshape(2, 128, D1).transpose(1, 0, 2)).astype(ml_dtypes.bfloat16)

    W2_ = np.asarray(W2, np.float32)
    a_s2 = np.asarray(att_src2, np.float32).reshape(OUT, 1)
    a_d2 = np.asarray(att_dst2, np.float32).reshape(OUT, 1)
    W2aug = np.concatenate([W2_, W2_ @ a_s2, W2_ @ a_d2], axis=1)     # [128,4]

    dummy1 = np.zeros((1, D1), dtype=np.float32)
    dummy1[0, HID:HID + HEADS] = PAD_ALPHA
    dummy2 = np.zeros((1, 4), dtype=np.float32)
    dummy2[0, 2] = PAD_ALPHA

    b1_b = np.tile(np.asarray(b1, np.float32)[None, :], (P, 1))
    b2_b = np.tile(np.asarray(b2, np.float32)[None, :], (P, 1))

    return dict(
        x_tiles=x_tiles, W1aug_t=W1aug_t, W2aug=W2aug,
        dummy1=dummy1, dummy2=dummy2, b1_b=b1_b, b2_b=b2_b,
        idx1=idx1, idx2=idx2, K_t=K_t, slab_off=slab_off,
        tot_slabs=tot_slabs, old_of_fid=old_of_fid,
        idx3=idx3, ixself=ixself, K3_tw=K3_tw, K3_t=K3_t, w_off=w_off,
        colbase=colbase, cols_t=cols_t, totcols=totcols,
    )


def _build_program(K_t, slab_off, tot_slabs, K3_tw, K3_t, w_off,
                   colbase, cols_t, totcols, dump_tables=False):
    import concourse.bass as bass
    import concourse.mybir as mybir
    from concourse.tile import TileContext
    from concourse.masks import make_identity

    f32 = mybir.dt.float32
    bf16 = mybir.dt.bfloat16
    i32 = mybir.dt.int32
    i16 = mybir.dt.int16
    AF = mybir.ActivationFunctionType
    OP = mybir.AluOpType

    nc = bass.Bass(target_bir_lowering=False)

    x_tiles = nc.dram_tensor("x_tiles", [N_ST, P, 2, P], bf16, kind="ExternalInput")
    w1aug = nc.dram_tensor("w1aug", [P, 2, D1], bf16, kind="ExternalInput")
    w2aug = nc.dram_tensor("w2aug", [HID, 4], f32, kind="ExternalInput")
    dummy1 = nc.dram_tensor("dummy1", [1, D1], f32, kind="ExternalInput")
    dummy2 = nc.dram_tensor("dummy2", [1, 4], f32, kind="ExternalInput")
    b1_b = nc.dram_tensor("b1_b", [P, HID], f32, kind="ExternalInput")
    b2_b = nc.dram_tensor("b2_b", [P, OUT], f32, kind="ExternalInput")
    idx1_in = nc.dram_tensor("idx1", [P, tot_slabs], i32, kind="ExternalInput")
    idx2_in = nc.dram_tensor("idx2", [P, tot_slabs], i32, kind="ExternalInput")
    out2 = nc.dram_tensor("out2", [ROWS_PER_CORE, OUT], f32, kind="ExternalOutput")
    if dump_tables:
        t1dump = nc.dram_tensor("t1dump", [N_PAD + 1, D1], f32, kind="ExternalOutput")
        t2dump = nc.dram_tensor("t2dump", [N_PAD, 4], f32, kind="ExternalOutput")

    with TileContext(nc) as tc:
        with tc.tile_pool(name="dram", bufs=1, space="DRAM") as dpool, \
             tc.tile_pool(name="const", bufs=1) as cpool, \
             tc.tile_pool(name="sb", bufs=3) as sb, \
             tc.tile_pool(name="gat", bufs=5) as gat, \
             tc.tile_pool(name="acc", bufs=2) as accp, \
             tc.tile_pool(name="ps", bufs=2, space="PSUM") as ps, \
             tc.tile_pool(name="p1", bufs=2) as p1pool:

            table1 = dpool.tile([N_PAD + 1, D1], f32)
            table2l = dpool.tile([ROWS_PER_CORE, 4], f32)
            table2 = dpool.tile([N_PAD, 4], f32, addr_space="Shared")

            # ---- constants ----
            w1_sb = cpool.tile([P, 2, D1], bf16)
            nc.sync.dma_start(out=w1_sb[:], in_=w1aug[:, :, :])
            w2_sb = cpool.tile([HID, 4], f32)
            nc.sync.dma_start(out=w2_sb[:], in_=w2aug[:, :])
            d1_sb = cpool.tile([1, D1], f32)
            nc.sync.dma_start(out=d1_sb[:], in_=dummy1[:, :])
            d2_sb = cpool.tile([1, 4], f32)
            nc.sync.dma_start(out=d2_sb[:], in_=dummy2[:, :])
            b1_sb = cpool.tile([P, HID], f32)
            nc.sync.dma_start(out=b1_sb[:], in_=b1_b[:, :])
            b2_sb = cpool.tile([P, OUT], f32)
            nc.sync.dma_start(out=b2_sb[:], in_=b2_b[:, :])
            ident = cpool.tile([P, P], f32)
            make_identity(nc, ident[:])

            nc.sync.dma_start(out=table1[N_PAD:N_PAD + 1, :], in_=d1_sb[:])
            nc.sync.dma_start(
                out=table2l[DUMMY2_LOCAL:DUMMY2_LOCAL + 1, :], in_=d2_sb[:])

            # ---- Phase 1: table1 (replicated on every core) ----
            # 8 tiles per DMA instruction (HWDGE issue cost ~0.6us dominates
            # phase 1 otherwise).
            SG = 8
            for s0 in range(0, N_ST, SG):
                xt = p1pool.tile([P, SG, 2, P], bf16, tag="xt")
                nc.sync.dma_start(
                    out=xt[:],
                    in_=x_tiles[s0:s0 + SG, :, :, :].rearrange(
                        "s p i n -> p s i n"))
                ht = p1pool.tile([P, SG, D1], f32, tag="ht")
                for j in range(SG):
                    hp = ps.tile([P, D1], f32, space="PSUM", tag="hp")
                    for i in range(2):
                        nc.tensor.matmul(out=hp[:], lhsT=xt[:, j, i, :],
                                         rhs=w1_sb[:, i, :],
                                         start=(i == 0), stop=(i == 1))
                    nc.vector.tensor_copy(ht[:, j, :], hp[:])
                nc.sync.dma_start(
                    out=table1[s0 * P:(s0 + SG) * P, :].rearrange(
                        "(s p) d -> p s d", s=SG),
                    in_=ht[:])

            # ---- preload the full idx arrays into SBUF once ----
            ix1_all = cpool.tile([P, tot_slabs], i32)
            nc.sync.dma_start(out=ix1_all[:], in_=idx1_in[:, :])
            ix2_all = cpool.tile([P, tot_slabs], i32)
            nc.sync.dma_start(out=ix2_all[:], in_=idx2_in[:, :])

            # ---- Phase 2: layer-1 aggregation -> table2_local ----
            for t in range(T_PER_CORE - 1):
                K = int(K_t[t])
                off = int(slab_off[t])
                gall = gat.tile([P, K, D1], f32, tag="gall")
                for k in range(K):
                    nc.gpsimd.indirect_dma_start(
                        out=gall[:, k, :], out_offset=None, in_=table1[:],
                        in_offset=bass.IndirectOffsetOnAxis(
                            ap=ix1_all[:, off + k:off + k + 1], axis=0))
                # e[p, h, k] = a_src(g_k) + a_dst(g_0)
                w_all = sb.tile([P, HEADS, K], f32, tag="w_all")
                nc.vector.tensor_tensor(
                    out=w_all[:],
                    in0=gall[:, :, HID:HID + HEADS].rearrange("p k h -> p h k"),
                    in1=gall[:, 0, HID + HEADS:HID + 2 * HEADS].to_broadcast(
                        [P, HEADS, K]),
                    op=OP.add)
                nc.scalar.activation(w_all[:], w_all[:], AF.Prelu, alpha=NEG)
                nc.scalar.activation(w_all[:], w_all[:], AF.Exp)
                den = sb.tile([P, HEADS], f32, tag="den")
                nc.vector.tensor_reduce(out=den[:], in_=w_all[:],
                                        axis=mybir.AxisListType.X, op=OP.add)
                rden = sb.tile([P, HEADS], f32, tag="rden")
                nc.vector.tensor_scalar(out=rden[:], in0=den[:], scalar1=1e-30,
                                        scalar2=None, op0=OP.add)
                nc.vector.reciprocal(rden[:], rden[:])
                u_all = accp.tile([P, K, HID], f32, tag="u_all")
                nc.vector.tensor_tensor(
                    out=u_all[:].rearrange("p k (h c) -> p k h c", h=HEADS),
                    in0=gall[:, :, 0:HID].rearrange("p k (h c) -> p k h c", h=HEADS),
                    in1=w_all[:].rearrange("p h k -> p k h").to_broadcast(
                        [P, K, HEADS, C1]),
                    op=OP.mult)
                acc = sb.tile([P, HID], f32, tag="acc")
                nc.vector.tensor_reduce(
                    out=acc[:], in_=u_all[:].rearrange("p k f -> p f k"),
                    axis=mybir.AxisListType.X, op=OP.add)
                h1 = sb.tile([P, HID], f32, tag="h1")
                nc.vector.tensor_tensor(
                    out=h1[:].rearrange("p (h c) -> p h c", h=HEADS),
                    in0=acc[:].rearrange("p (h c) -> p h c", h=HEADS),
                    in1=rden[:].to_broadcast([P, HEADS, C1]),
                    op=OP.mult)
                nc.vector.tensor_add(h1[:], h1[:], b1_sb[:])
                # elu
                m = sb.tile([P, HID], f32, tag="m")
                nc.vector.tensor_scalar(out=m[:], in0=h1[:], scalar1=0.0,
                                        scalar2=None, op0=OP.min)
                em = sb.tile([P, HID], f32, tag="em")
                nc.scalar.activation(em[:], m[:], AF.Exp)
                nc.vector.tensor_scalar(out=h1[:], in0=h1[:], scalar1=0.0,
                                        scalar2=None, op0=OP.max)
                nc.vector.tensor_add(h1[:], h1[:], em[:])
                nc.vector.tensor_scalar(out=h1[:], in0=h1[:], scalar1=1.0,
                                        scalar2=None, op0=OP.subtract)
                # table2 row block = h1 @ W2aug (via PE transpose)
                h1tp = ps.tile([P, P], f32, space="PSUM", tag="h1tp")
                nc.tensor.transpose(out=h1tp[:], in_=h1[:], identity=ident[:])
                h1t = sb.tile([P, P], f32, tag="h1t")
                nc.vector.tensor_copy(h1t[:], h1tp[:])
                t2p = ps.tile([P, 4], f32, space="PSUM", tag="t2p")
                nc.tensor.matmul(out=t2p[:], lhsT=h1t[:], rhs=w2_sb[:],
                                 start=True, stop=True)
                t2 = sb.tile([P, 4], f32, tag="t2")
                nc.vector.tensor_copy(t2[:], t2p[:])
                nc.sync.dma_start(out=table2l[t * P:(t + 1) * P, :], in_=t2[:])

            # ---- AllGather table2 ----
            nc.gpsimd.collective_compute(
                "AllGather", mybir.AluOpType.bypass,
                replica_groups=[list(range(NCORES))],
                ins=[table2l[:, :]], outs=[table2[:, :]])

            # ---- Phase 3: layer-2 aggregation -> out2 ----
            for t in range(T_PER_CORE - 1):
                K = int(K_t[t])
                off = int(slab_off[t])
                gall = gat.tile([P, K, 4], f32, tag="gall3")
                for k in range(K):
                    nc.gpsimd.indirect_dma_start(
                        out=gall[:, k, :], out_offset=None, in_=table2[:],
                        in_offset=bass.IndirectOffsetOnAxis(
                            ap=ix2_all[:, off + k:off + k + 1], axis=0))
                w_all = sb.tile([P, K], f32, tag="w_all3")
                nc.vector.tensor_tensor(
                    out=w_all[:],
                    in0=gall[:, :, 2:3].rearrange("p k o -> p (k o)"),
                    in1=gall[:, 0, 3:4].to_broadcast([P, K]),
                    op=OP.add)
                nc.scalar.activation(w_all[:], w_all[:], AF.Prelu, alpha=NEG)
                nc.scalar.activation(w_all[:], w_all[:], AF.Exp)
                den = sb.tile([P, 1], f32, tag="den3")
                nc.vector.tensor_reduce(out=den[:], in_=w_all[:],
                                        axis=mybir.AxisListType.X, op=OP.add)
                u_all = sb.tile([P, K, OUT], f32, tag="u_all3")
                nc.vector.tensor_tensor(
                    out=u_all[:],
                    in0=gall[:, :, 0:OUT],
                    in1=w_all[:].to_broadcast([P, K, OUT]),
                    op=OP.mult)
                acc = sb.tile([P, OUT], f32, tag="acc3")
                nc.vector.tensor_reduce(
                    out=acc[:], in_=u_all[:].rearrange("p k o -> p o k"),
                    axis=mybir.AxisListType.X, op=OP.add)
                rden = sb.tile([P, 1], f32, tag="rden3")
                nc.vector.tensor_scalar(out=rden[:], in0=den[:], scalar1=1e-30,
                                        scalar2=None, op0=OP.add)
                nc.vector.reciprocal(rden[:], rden[:])
                o2 = sb.tile([P, OUT], f32, tag="o2")
                nc.vector.tensor_tensor(out=o2[:], in0=acc[:],
                                        in1=rden[:].to_broadcast([P, OUT]),
                                        op=OP.mult)
                nc.vector.tensor_add(o2[:], o2[:], b2_sb[:])
                nc.sync.dma_start(out=out2[t * P:(t + 1) * P, :], in_=o2[:])

            if dump_tables:
                nc.sync.dma_start(out=t1dump[:, :], in_=table1[:])
                nc.sync.dma_start(out=t2dump[:, :], in_=table2[:])

    _split_waits(nc)
    return nc


def _split_waits(nc, max_waits=1):
    """This walrus build allows one sync-wait slot per instruction; hoist
    excess waits onto same-engine InstNoOp carriers inserted just before."""
    import concourse.mybir as mybir
    ctr = 0
    for fn in nc.m.functions:
        for bb in fn.blocks:
            out = []
            changed = False
            for inst in bb.instructions:
                si = inst.sync_info
                if si is not None and len(si.on_wait) > max_waits:
                    waits = list(si.on_wait)
                    extra, keep = waits[:-max_waits], waits[-max_waits:]
                    for i in range(0, len(extra), max_waits):
                        ctr += 1
                        nop = mybir.InstNoOp(name=f"waitfix-{ctr}", ins=[], outs=[])
                        nop.engine = inst.engine
                        nop.sync_info = mybir.SyncInfo(
                            on_wait=extra[i:i + max_waits], on_update=[])
                        out.append(nop)
                    si.on_wait = keep
                    inst.sync_info = si
                    changed = True
                out.append(inst)
            if changed:
                bb.instructions = out
    return ctr


def kernel(x, edge_index, W1, att_src1, att_dst1, b1, W2, att_src2, att_dst2, b2):
    from concourse.bass_utils import run_bass_kernel_spmd

    prep = _host_prep(x, edge_index, W1, att_src1, att_dst1, b1,
                      W2, att_src2, att_dst2, b2)
    nc = _build_program(prep["K_t"], prep["slab_off"], prep["tot_slabs"],
                        prep["K3_tw"], prep["K3_t"], prep["w_off"],
                        prep["colbase"], prep["cols_t"], prep["totcols"])

    in_maps = []
    for c in range(NCORES):
        in_maps.append({
            "x_tiles": prep["x_tiles"],
            "w1aug": prep["W1aug_t"],
            "w2aug": prep["W2aug"],
            "dummy1": prep["dummy1"],
            "dummy2": prep["dummy2"],
            "b1_b": prep["b1_b"],
            "b2_b": prep["b2_b"],
            "idx1": np.ascontiguousarray(prep["idx1"][c]),
            "idx2": np.ascontiguousarray(prep["idx2"][c]),
        })

    trace = bool(os.environ.get("GAT_TRACE"))
    if trace:
        _install_ntff_shim()
    r = run_bass_kernel_spmd(nc, in_maps, core_ids=list(range(NCORES)),
                             trace=trace)
    if trace and r.exec_time_ns:
        print(f"HW exec time: {r.exec_time_ns} ns")

    old_of_fid = prep["old_of_fid"]
    out = np.zeros((N, OUT), dtype=np.float32)
    for c in range(NCORES):
        fid0 = c * ROWS_PER_CORE
        olds = old_of_fid[fid0:fid0 + ROWS_PER_CORE]
        m = olds >= 0
        out[olds[m]] = r.results[c]["out2"][m]
    return out


def _install_ntff_shim():
    """The image's antenv lacks axon_hooks; recreate it so trace=True works."""
    import sys, types
    if "antenv.axon_hooks" in sys.modules:
        return
    sys.path.insert(0, "/root/.axon_site/trn_agent_boot")
    try:
        import trn_boot
        hook = trn_boot._ntff_profile_via_ctypes("/opt/axon/libaxon_pjrt.so")
    except Exception:
        hook = None
    mod = types.ModuleType("antenv.axon_hooks")
    mod.get_axon_ntff_profile_hook = lambda: hook
    mod.set_axon_ntff_profile_hook = lambda h: None
    sys.modules["antenv.axon_hooks"] = mod



# revision 45
# speedup vs baseline: 1.0027x; 1.0027x over previous
"""GAT (2-layer: 4 heads -> 1 head) on 8 trn2 NeuronCores.

Strategy (dst-partitioned slot design):
 - Relabel nodes by in-degree (ascending) into core-major blocks:
   final id = c*(T*128) + t*128 + d, where sorted 128-node block s = 8t + c.
   Each core owns T = 99 blocks (98 data + 1 all-dummy tail block whose slot
   12544 holds the table2 "dummy row" at the same local address on every
   core). Slab structure K_t is shared by all cores -> one SPMD program.
 - Per super-tile, edges live in "slabs": slab k holds edge k of every dst
   (one edge per partition). Self-loops pinned to slab 0, so the slab-0
   gather also delivers each dst's own table row (alpha_dst for free).
 - Phase 1 (replicated): table1[n] = x[n] @ [W1 | W1@A1s | W1@A1d] (136 f32).
 - Phase 2: per slab, [P,1] indirect-gather of table1[src]; per-partition
   softmax accumulation (exp without max-subtraction -- padded slots gather
   a dummy row with alpha_src=-1e4, so w = exp(leaky(-1e4+a_d)) == 0).
   Epilogue computes h1 then table2_local[t*128+d] = [h2 | a_s2 | a_d2].
 - AllGather table2_local (12672x4) -> table2_full (101376x4, Shared).
 - Phase 3: same slab loop over 16-byte table2 rows -> out2 [12672, 2].
"""
import os
import numpy as np

N = 100000
E = 1600000
IN_DIM = 256
HID = 128
HEADS = 4
C1 = HID // HEADS
OUT = 2
NEG = 0.2
P = 128
NCORES = 8
N_ST_DATA = 784                   # data blocks (ceil(100000/128) = 782 -> 784 for /8)
T_PER_CORE = N_ST_DATA // NCORES + 1   # 99 (incl. all-dummy tail block)
N_ST = T_PER_CORE * NCORES        # 792
N_PAD = N_ST * P                  # 101376
ROWS_PER_CORE = T_PER_CORE * P    # 12672
DUMMY1 = N_PAD                    # table1 dummy row (extra row)
DUMMY2_LOCAL = (T_PER_CORE - 1) * P   # local row 12544 in every core's block
PAD_ALPHA = -1.0e4
D1 = HID + 2 * HEADS              # 136


def _host_prep(x, edge_index, W1, att_src1, att_dst1, b1, W2, att_src2, att_dst2, b2):
    src0 = np.asarray(edge_index[0], dtype=np.int64)
    dst0 = np.asarray(edge_index[1], dtype=np.int64)
    loops = np.arange(N, dtype=np.int64)
    src = np.concatenate([src0, loops])
    dst = np.concatenate([dst0, loops])

    deg = np.bincount(dst, minlength=N)
    order = np.argsort(deg, kind="stable")        # sorted position -> old id

    # sorted position p = s*128+d, block s = 8t+c  ->  fid = (c*T + t)*128 + d
    n_data = N_ST_DATA * P                        # 100352 sorted slots
    p_ar = np.arange(n_data, dtype=np.int64)
    s_ar = p_ar // P
    d_ar = p_ar % P
    t_ar = s_ar // NCORES
    c_ar = s_ar % NCORES
    fid_of_sorted = (c_ar * T_PER_CORE + t_ar) * P + d_ar   # [n_data]

    fid_of_old = np.empty(N, dtype=np.int64)
    fid_of_old[order] = fid_of_sorted[:N]
    old_of_fid = np.full(N_PAD, -1, dtype=np.int64)
    old_of_fid[fid_of_old] = np.arange(N)

    nsrc = fid_of_old[src]
    ndst = fid_of_old[dst]
    ndeg = np.zeros(N_PAD, dtype=np.int64)
    ndeg[fid_of_old] = deg

    K_ct = ndeg.reshape(NCORES, T_PER_CORE, P).max(-1)      # [8, 99]
    K_t = np.maximum(K_ct.max(0), 1)                        # [99]
    tot_slabs = int(K_t.sum())
    slab_off = np.concatenate([[0], np.cumsum(K_t)]).astype(np.int64)

    # slot index k of each edge within its dst; self-loop forced to k=0
    notself = (nsrc != ndst).astype(np.int8)
    key = np.lexsort((notself, ndst))
    ds_ = ndst[key]
    ss_ = nsrc[key]
    counts = np.bincount(ds_, minlength=N_PAD)
    run_start = np.zeros(N_PAD + 1, dtype=np.int64)
    run_start[1:] = np.cumsum(counts)
    kk = np.arange(ds_.size, dtype=np.int64) - run_start[ds_]

    blk = ds_ // P
    core = blk // T_PER_CORE
    t_idx = blk % T_PER_CORE
    d_rel = ds_ % P
    col = slab_off[t_idx] + kk

    idx1 = np.full((NCORES, P, tot_slabs), DUMMY1, dtype=np.int32)
    idx1[core, d_rel, col] = ss_.astype(np.int32)
    idx2 = idx1.copy()
    for c in range(NCORES):
        pad_val = c * ROWS_PER_CORE + DUMMY2_LOCAL
        m = idx2[c] == DUMMY1
        idx2[c][m] = pad_val

    # ---- phase-3 windowed dma_gather structures (currently unused) ----
    # table2big rows (256B bf16) indexed by fid; 4 windows of WS rows with
    # int16 window-local indices.  Self-loops handled separately (slab-0
    # style indirect gather on the compact table2), so windows hold only
    # non-self edges.  Window dummy = each window's resident core-tail
    # dummy row (alpha_src = -1e4 -> weight 0).
    WS = 32768
    NW = 4
    wdum = np.array([12544, 37888, 75904, 101248], dtype=np.int64)
    assert all(wdum // WS == np.arange(NW))
    ns_mask = ss_ != ds_
    dsn = ds_[ns_mask]
    ssn = ss_[ns_mask]
    wn = np.minimum(ssn // WS, NW - 1)
    blk_n = dsn // P
    core_n = blk_n // T_PER_CORE
    t_n = blk_n % T_PER_CORE
    p_n = dsn % P
    # per (core, t, p, w) counts and slot index within that bucket
    keyw = np.lexsort((ssn, wn, dsn))
    dsw = dsn[keyw]; ssw = ssn[keyw]; www = wn[keyw]
    bucket = dsw * NW + www
    cntw = np.bincount(bucket, minlength=N_PAD * NW)
    rs = np.zeros(N_PAD * NW + 1, dtype=np.int64)
    rs[1:] = np.cumsum(cntw)
    kw = np.arange(dsw.size, dtype=np.int64) - rs[bucket]
    cntw4 = cntw.reshape(N_PAD, NW)
    # K3_tw[t, w] = max over cores and dsts of row t (shared SPMD shape)
    K3_tw = cntw4.reshape(NCORES, T_PER_CORE, P, NW).max(2).max(0)  # [T, W]
    K3_tw = K3_tw[:T_PER_CORE - 1]          # data tiles only
    K3_t = K3_tw.sum(1)                      # [98]
    w_off = np.zeros((T_PER_CORE - 1, NW), dtype=np.int64)
    w_off[:, 1:] = np.cumsum(K3_tw, axis=1)[:, :-1]
    # flat int16 idx grid per core: for (t, w): [K3_tw, 128] slots
    # (k-major), value = window-local src fid (or window dummy)
    cols_t = (K3_tw * P // 16).sum(1)        # int16 cols per tile
    colbase = np.zeros(T_PER_CORE, dtype=np.int64)
    colbase[1:] = np.cumsum(cols_t)
    totcols = int(colbase[T_PER_CORE - 1])
    idx3 = np.empty((NCORES, 16, totcols), dtype=np.int16)
    for c in range(NCORES):
        # per-(t, w) slot grid, default = window-local dummy
        grid = {(t, w): np.full((int(K3_tw[t, w]), P), wdum[w] - w * WS,
                                dtype=np.int64)
                for t in range(T_PER_CORE - 1) for w in range(NW)}
        m_c = core_n[keyw] == c
        tt = t_n[keyw][m_c]; pp = p_n[keyw][m_c]
        wwc = www[m_c]; kkc = kw[m_c]; ssc = ssw[m_c]
        for t in range(T_PER_CORE - 1):
            mt = tt == t
            for w in range(NW):
                mw = mt & (wwc == w)
                g = grid[(t, w)]
                g[kkc[mw], pp[mw]] = ssc[mw] - w * WS
        pieces = []
        for t in range(T_PER_CORE - 1):
            for w in range(NW):
                g = grid[(t, w)]                     # [Kw, 128] k-major
                if g.size == 0:
                    continue
                fl = g.reshape(-1)                    # flat i = k*128+p
                pieces.append(fl.reshape(-1, 16).T)   # [16, n/16]
        idx3[c] = np.concatenate(pieces, axis=1).astype(np.int16)
    # self-loop (slab-0) indices into compact table2 per (core, tile)
    ixself = np.empty((NCORES, P, T_PER_CORE - 1), dtype=np.int32)
    for c in range(NCORES):
        for t in range(T_PER_CORE - 1):
            base = (c * T_PER_CORE + t) * P
            fids = np.arange(base, base + P)
            real = old_of_fid[fids] >= 0
            v = np.where(real, fids, c * ROWS_PER_CORE + DUMMY2_LOCAL)
            ixself[c, :, t] = v.astype(np.int32)

    # x tiles [792, 128, 2, 128] bf16: [s, p, i, n] = xp[s*128+n, i*128+p]
    # (partition-major contiguous so each block load is one flat 64KB DMA)
    import ml_dtypes
    xp = np.zeros((N_PAD, IN_DIM), dtype=np.float32)
    xp[fid_of_old] = np.asarray(x, dtype=np.float32)
    x_tiles = np.ascontiguousarray(
        xp.reshape(N_ST, P, 2, P).transpose(0, 3, 2, 1)).astype(ml_dtypes.bfloat16)

    A1s = np.zeros((HID, HEADS), dtype=np.float32)
    A1d = np.zeros((HID, HEADS), dtype=np.float32)
    for h in range(HEADS):
        A1s[h * C1:(h + 1) * C1, h] = np.asarray(att_src1, np.float32)[h]
        A1d[h * C1:(h + 1) * C1, h] = np.asarray(att_dst1, np.float32)[h]
    W1_ = np.asarray(W1, np.float32)
    W1aug = np.concatenate([W1_, W1_ @ A1s, W1_ @ A1d], axis=1)       # [256,136]
    W1aug_t = np.ascontiguousarray(
        W1aug.reshape(2, 128, D1).transpose(1, 0, 2)).astype(ml_dtypes.bfloat16)

    W2_ = np.asarray(W2, np.float32)
    a_s2 = np.asarray(att_src2, np.float32).reshape(OUT, 1)
    a_d2 = np.asarray(att_dst2, np.float32).reshape(OUT, 1)
    W2aug = np.concatenate([W2_, W2_ @ a_s2, W2_ @ a_d2], axis=1)     # [128,4]

    dummy1 = np.zeros((1, D1), dtype=np.float32)
    dummy1[0, HID:HID + HEADS] = PAD_ALPHA
    dummy2 = np.zeros((1, 4), dtype=np.float32)
    dummy2[0, 2] = PAD_ALPHA

    b1_b = np.tile(np.asarray(b1, np.float32)[None, :], (P, 1))
    b2_b = np.tile(np.asarray(b2, np.float32)[None, :], (P, 1))

    return dict(
        x_tiles=x_tiles, W1aug_t=W1aug_t, W2aug=W2aug,
        dummy1=dummy1, dummy2=dummy2, b1_b=b1_b, b2_b=b2_b,
        idx1=idx1, idx2=idx2, K_t=K_t, slab_off=slab_off,
        tot_slabs=tot_slabs, old_of_fid=old_of_fid,
        idx3=idx3, ixself=ixself, K3_tw=K3_tw, K3_t=K3_t, w_off=w_off,
        colbase=colbase, cols_t=cols_t, totcols=totcols,
    )


def _build_program(K_t, slab_off, tot_slabs, K3_tw, K3_t, w_off,
                   colbase, cols_t, totcols, dump_tables=False):
    import concourse.bass as bass
    import concourse.mybir as mybir
    from concourse.tile import TileContext
    from concourse.masks import make_identity

    f32 = mybir.dt.float32
    bf16 = mybir.dt.bfloat16
    i32 = mybir.dt.int32
    i16 = mybir.dt.int16
    AF = mybir.ActivationFunctionType
    OP = mybir.AluOpType

    nc = bass.Bass(target_bir_lowering=False)

    x_tiles = nc.dram_tensor("x_tiles", [N_ST, P, 2, P], bf16, kind="ExternalInput")
    w1aug = nc.dram_tensor("w1aug", [P, 2, D1], bf16, kind="ExternalInput")
    w2aug = nc.dram_tensor("w2aug", [HID, 4], f32, kind="ExternalInput")
    dummy1 = nc.dram_tensor("dummy1", [1, D1], f32, kind="ExternalInput")
    dummy2 = nc.dram_tensor("dummy2", [1, 4], f32, kind="ExternalInput")
    b1_b = nc.dram_tensor("b1_b", [P, HID], f32, kind="ExternalInput")
    b2_b = nc.dram_tensor("b2_b", [P, OUT], f32, kind="ExternalInput")
    idx1_in = nc.dram_tensor("idx1", [P, tot_slabs], i32, kind="ExternalInput")
    idx2_in = nc.dram_tensor("idx2", [P, tot_slabs], i32, kind="ExternalInput")
    out2 = nc.dram_tensor("out2", [ROWS_PER_CORE, OUT], f32, kind="ExternalOutput")
    if dump_tables:
        t1dump = nc.dram_tensor("t1dump", [N_PAD + 1, D1], f32, kind="ExternalOutput")
        t2dump = nc.dram_tensor("t2dump", [N_PAD, 4], f32, kind="ExternalOutput")

    with TileContext(nc) as tc:
        with tc.tile_pool(name="dram", bufs=1, space="DRAM") as dpool, \
             tc.tile_pool(name="const", bufs=1) as cpool, \
             tc.tile_pool(name="sb", bufs=3) as sb, \
             tc.tile_pool(name="gat", bufs=5) as gat, \
             tc.tile_pool(name="acc", bufs=2) as accp, \
             tc.tile_pool(name="ps", bufs=2, space="PSUM") as ps, \
             tc.tile_pool(name="p1", bufs=2) as p1pool:

            table1 = dpool.tile([N_PAD + 1, D1], f32)
            table2l = dpool.tile([ROWS_PER_CORE, 4], f32)
            table2 = dpool.tile([N_PAD, 4], f32, addr_space="Shared")

            # ---- constants ----
            w1_sb = cpool.tile([P, 2, D1], bf16)
            nc.sync.dma_start(out=w1_sb[:], in_=w1aug[:, :, :])
            w2_sb = cpool.tile([HID, 4], f32)
            nc.sync.dma_start(out=w2_sb[:], in_=w2aug[:, :])
            d1_sb = cpool.tile([1, D1], f32)
            nc.sync.dma_start(out=d1_sb[:], in_=dummy1[:, :])
            d2_sb = cpool.tile([1, 4], f32)
            nc.sync.dma_start(out=d2_sb[:], in_=dummy2[:, :])
            b1_sb = cpool.tile([P, HID], f32)
            nc.sync.dma_start(out=b1_sb[:], in_=b1_b[:, :])
            b2_sb = cpool.tile([P, OUT], f32)
            nc.sync.dma_start(out=b2_sb[:], in_=b2_b[:, :])
            ident = cpool.tile([P, P], f32)
            make_identity(nc, ident[:])

            nc.sync.dma_start(out=table1[N_PAD:N_PAD + 1, :], in_=d1_sb[:])
            nc.sync.dma_start(
                out=table2l[DUMMY2_LOCAL:DUMMY2_LOCAL + 1, :], in_=d2_sb[:])

            # ---- Phase 1: table1 (replicated on every core) ----
            # 8 tiles per DMA instruction (HWDGE issue cost ~0.6us dominates
            # phase 1 otherwise).
            SG = 8
            for s0 in range(0, N_ST, SG):
                xt = p1pool.tile([P, SG, 2, P], bf16, tag="xt")
                nc.sync.dma_start(
                    out=xt[:],
                    in_=x_tiles[s0:s0 + SG, :, :, :].rearrange(
                        "s p i n -> p s i n"))
                ht = p1pool.tile([P, SG, D1], f32, tag="ht")
                for j in range(SG):
                    hp = ps.tile([P, D1], f32, space="PSUM", tag="hp")
                    for i in range(2):
                        nc.tensor.matmul(out=hp[:], lhsT=xt[:, j, i, :],
                                         rhs=w1_sb[:, i, :],
                                         start=(i == 0), stop=(i == 1))
                    nc.vector.tensor_copy(ht[:, j, :], hp[:])
                nc.sync.dma_start(
                    out=table1[s0 * P:(s0 + SG) * P, :].rearrange(
                        "(s p) d -> p s d", s=SG),
                    in_=ht[:])

            # ---- preload the full idx arrays into SBUF once ----
            ix1_all = cpool.tile([P, tot_slabs], i32)
            nc.sync.dma_start(out=ix1_all[:], in_=idx1_in[:, :])
            ix2_all = cpool.tile([P, tot_slabs], i32)
            nc.sync.dma_start(out=ix2_all[:], in_=idx2_in[:, :])

            # ---- Phase 2: layer-1 aggregation -> table2_local ----
            for t in range(T_PER_CORE - 1):
                K = int(K_t[t])
                off = int(slab_off[t])
                gall = gat.tile([P, K, D1], f32, tag="gall")
                for k in range(K):
                    nc.gpsimd.indirect_dma_start(
                        out=gall[:, k, :], out_offset=None, in_=table1[:],
                        in_offset=bass.IndirectOffsetOnAxis(
                            ap=ix1_all[:, off + k:off + k + 1], axis=0))
                # e[p, h, k] = a_src(g_k) + a_dst(g_0)
                w_all = sb.tile([P, HEADS, K], f32, tag="w_all")
                nc.vector.tensor_tensor(
                    out=w_all[:],
                    in0=gall[:, :, HID:HID + HEADS].rearrange("p k h -> p h k"),
                    in1=gall[:, 0, HID + HEADS:HID + 2 * HEADS].to_broadcast(
                        [P, HEADS, K]),
                    op=OP.add)
                nc.scalar.activation(w_all[:], w_all[:], AF.Prelu, alpha=NEG)
                nc.scalar.activation(w_all[:], w_all[:], AF.Exp)
                den = sb.tile([P, HEADS], f32, tag="den")
                nc.vector.tensor_reduce(out=den[:], in_=w_all[:],
                                        axis=mybir.AxisListType.X, op=OP.add)
                rden = sb.tile([P, HEADS], f32, tag="rden")
                nc.vector.tensor_scalar(out=rden[:], in0=den[:], scalar1=1e-30,
                                        scalar2=None, op0=OP.add)
                nc.vector.reciprocal(rden[:], rden[:])
                u_all = accp.tile([P, K, HID], f32, tag="u_all")
                nc.vector.tensor_tensor(
                    out=u_all[:].rearrange("p k (h c) -> p k h c", h=HEADS),
                    in0=gall[:, :, 0:HID].rearrange("p k (h c) -> p k h c", h=HEADS),
                    in1=w_all[:].rearrange("p h k -> p k h").to_broadcast(
                        [P, K, HEADS, C1]),
                    op=OP.mult)
                acc = sb.tile([P, HID], f32, tag="acc")
                nc.vector.tensor_reduce(
                    out=acc[:], in_=u_all[:].rearrange("p k f -> p f k"),
                    axis=mybir.AxisListType.X, op=OP.add)
                h1 = sb.tile([P, HID], f32, tag="h1")
                nc.vector.tensor_tensor(
                    out=h1[:].rearrange("p (h c) -> p h c", h=HEADS),
                    in0=acc[:].rearrange("p (h c) -> p h c", h=HEADS),
                    in1=rden[:].to_broadcast([P, HEADS, C1]),
                    op=OP.mult)
                nc.vector.tensor_add(h1[:], h1[:], b1_sb[:])
                # elu
                m = sb.tile([P, HID], f32, tag="m")
                nc.vector.tensor_scalar(out=m[:], in0=h1[:], scalar1=0.0,
                                        scalar2=None, op0=OP.min)
                em = sb.tile([P, HID], f32, tag="em")
                nc.scalar.activation(em[:], m[:], AF.Exp)
                nc.vector.tensor_scalar(out=h1[:], in0=h1[:], scalar1=0.0,
                                        scalar2=None, op0=OP.max)
                nc.vector.tensor_add(h1[:], h1[:], em[:])
                nc.vector.tensor_scalar(out=h1[:], in0=h1[:], scalar1=1.0,
                                        scalar2=None, op0=OP.subtract)
                # table2 row block = h1 @ W2aug (via PE transpose)
                h1tp = ps.tile([P, P], f32, space="PSUM", tag="h1tp")
                nc.tensor.transpose(out=h1tp[:], in_=h1[:], identity=ident[:])
                h1t = sb.tile([P, P], f32, tag="h1t")
                nc.vector.tensor_copy(h1t[:], h1tp[:])
                t2p = ps.tile([P, 4], f32, space="PSUM", tag="t2p")
                nc.tensor.matmul(out=t2p[:], lhsT=h1t[:], rhs=w2_sb[:],
                                 start=True, stop=True)
                t2 = sb.tile([P, 4], f32, tag="t2")
                nc.vector.tensor_copy(t2[:], t2p[:])
                nc.sync.dma_start(out=table2l[t * P:(t + 1) * P, :], in_=t2[:])

            # ---- AllGather table2 ----
            nc.gpsimd.collective_compute(
                "AllGather", mybir.AluOpType.bypass,
                replica_groups=[list(range(NCORES))],
                ins=[table2l[:, :]], outs=[table2[:, :]])

            # ---- Phase 3: layer-2 aggregation -> out2 ----
            for t in range(T_PER_CORE - 1):
                K = int(K_t[t])
                off = int(slab_off[t])
                gall = gat.tile([P, K, 4], f32, tag="gall3")
                for k in range(K):
                    nc.gpsimd.indirect_dma_start(
                        out=gall[:, k, :], out_offset=None, in_=table2[:],
                        in_offset=bass.IndirectOffsetOnAxis(
                            ap=ix2_all[:, off + k:off + k + 1], axis=0))
                w_all = sb.tile([P, K], f32, tag="w_all3")
                nc.vector.tensor_tensor(
                    out=w_all[:],
                    in0=gall[:, :, 2:3].rearrange("p k o -> p (k o)"),
                    in1=gall[:, 0, 3:4].to_broadcast([P, K]),
                    op=OP.add)
                nc.scalar.activation(w_all[:], w_all[:], AF.Prelu, alpha=NEG)
                nc.scalar.activation(w_all[:], w_all[:], AF.Exp)
                den = sb.tile([P, 1], f32, tag="den3")
                nc.vector.tensor_reduce(out=den[:], in_=w_all[:],
                                        axis=mybir.AxisListType.X, op=OP.add)
                u_all = sb.tile([P, K, OUT], f32, tag="u_all3")
                nc.vector.tensor_tensor(
                    out=u_all[:],
                    in0=gall[:, :, 0:OUT],
                    in1=w_all[:].to_broadcast([P, K, OUT]),
                    op=OP.mult)
                acc = sb.tile([P, OUT], f32, tag="acc3")
                nc.vector.tensor_reduce(
                    out=acc[:], in_=u_all[:].rearrange("p k o -> p o k"),
                    axis=mybir.AxisListType.X, op=OP.add)
                rden = sb.tile([P, 1], f32, tag="rden3")
                nc.vector.tensor_scalar(out=rden[:], in0=den[:], scalar1=1e-30,
                                        scalar2=None, op0=OP.add)
                nc.vector.reciprocal(rden[:], rden[:])
                o2 = sb.tile([P, OUT], f32, tag="o2")
                nc.vector.tensor_tensor(out=o2[:], in0=acc[:],
                                        in1=rden[:].to_broadcast([P, OUT]),
                                        op=OP.mult)
                nc.vector.tensor_add(o2[:], o2[:], b2_sb[:])
                nc.sync.dma_start(out=out2[t * P:(t + 1) * P, :], in_=o2[:])

            if dump_tables:
                nc.sync.dma_start(out=t1dump[:, :], in_=table1[:])
                nc.sync.dma_start(out=t2dump[:, :], in_=table2[:])

    _split_waits(nc)
    return nc


def _split_waits(nc, max_waits=1):
    """This walrus build allows one sync-wait slot per instruction; hoist
    excess waits onto same-engine InstNoOp carriers inserted just before."""
    import concourse.mybir as mybir
    ctr = 0
    for fn in nc.m.functions:
        for bb in fn.blocks:
            out = []
            changed = False
            for inst in bb.instructions:
                si = inst.sync_info
                if si is not None and len(si.on_wait) > max_waits:
                    waits = list(si.on_wait)
                    extra, keep = waits[:-max_waits], waits[-max_waits:]
                    for i in range(0, len(extra), max_waits):
                        ctr += 1
                        nop = mybir.InstNoOp(name=f"waitfix-{ctr}", ins=[], outs=[])
                        nop.engine = inst.engine
                        nop.sync_info = mybir.SyncInfo(
                            on_wait=extra[i:i + max_waits], on_update=[])
                        out.append(nop)
                    si.on_wait = keep
                    inst.sync_info = si
                    changed = True
                out.append(inst)
            if changed:
                bb.instructions = out
    return ctr


def kernel(x, edge_index, W1, att_src1, att_dst1, b1, W2, att_src2, att_dst2, b2):
    from concourse.bass_utils import run_bass_kernel_spmd

    prep = _host_prep(x, edge_index, W1, att_src1, att_dst1, b1,
                      W2, att_src2, att_dst2, b2)
    nc = _build_program(prep["K_t"], prep["slab_off"], prep["tot_slabs"],
                        prep["K3_tw"], prep["K3_t"], prep["w_off"],
                        prep["colbase"], prep["cols_t"], prep["totcols"])

    in_maps = []
    for c in range(NCORES):
        in_maps.append({
            "x_tiles": prep["x_tiles"],
            "w1aug": prep["W1aug_t"],
            "w2aug": prep["W2aug"],
            "dummy1": prep["dummy1"],
            "dummy2": prep["dummy2"],
            "b1_b": prep["b1_b"],
            "b2_b": prep["b2_b"],
            "idx1": np.ascontiguousarray(prep["idx1"][c]),
            "idx2": np.ascontiguousarray(prep["idx2"][c]),
        })

    trace = bool(os.environ.get("GAT_TRACE"))
    if trace:
        _install_ntff_shim()
    r = run_bass_kernel_spmd(nc, in_maps, core_ids=list(range(NCORES)),
                             trace=trace)
    if trace and r.exec_time_ns:
        print(f"HW exec time: {r.exec_time_ns} ns")

    old_of_fid = prep["old_of_fid"]
    out = np.zeros((N, OUT), dtype=np.float32)
    for c in range(NCORES):
        fid0 = c * ROWS_PER_CORE
        olds = old_of_fid[fid0:fid0 + ROWS_PER_CORE]
        m = olds >= 0
        out[olds[m]] = r.results[c]["out2"][m]
    return out


def _install_ntff_shim():
    """The image's antenv lacks axon_hooks; recreate it so trace=True works."""
    import sys, types
    if "antenv.axon_hooks" in sys.modules:
        return
    sys.path.insert(0, "/root/.axon_site/trn_agent_boot")
    try:
        import trn_boot
        hook = trn_boot._ntff_profile_via_ctypes("/opt/axon/libaxon_pjrt.so")
    except Exception:
        hook = None
    mod = types.ModuleType("antenv.axon_hooks")
    mod.get_axon_ntff_profile_hook = lambda: hook
    mod.set_axon_ntff_profile_hook = lambda h: None
    sys.modules["antenv.axon_hooks"] = mod



# revision 46
# speedup vs baseline: 1.0254x; 1.0227x over previous
"""GAT (2-layer: 4 heads -> 1 head) on 8 trn2 NeuronCores.

Strategy (dst-partitioned slot design):
 - Relabel nodes by in-degree (ascending) into core-major blocks:
   final id = c*(T*128) + t*128 + d, where sorted 128-node block s = 8t + c.
   Each core owns T = 99 blocks (98 data + 1 all-dummy tail block whose slot
   12544 holds the table2 "dummy row" at the same local address on every
   core). Slab structure K_t is shared by all cores -> one SPMD program.
 - Per super-tile, edges live in "slabs": slab k holds edge k of every dst
   (one edge per partition). Self-loops pinned to slab 0, so the slab-0
   gather also delivers each dst's own table row (alpha_dst for free).
 - Phase 1 (replicated): table1[n] = x[n] @ [W1 | W1@A1s | W1@A1d] (136 f32).
 - Phase 2: per slab, [P,1] indirect-gather of table1[src]; per-partition
   softmax accumulation (exp without max-subtraction -- padded slots gather
   a dummy row with alpha_src=-1e4, so w = exp(leaky(-1e4+a_d)) == 0).
   Epilogue computes h1 then table2_local[t*128+d] = [h2 | a_s2 | a_d2].
 - AllGather table2_local (12672x4) -> table2_full (101376x4, Shared).
 - Phase 3: same slab loop over 16-byte table2 rows -> out2 [12672, 2].
"""
import os
import numpy as np

N = 100000
E = 1600000
IN_DIM = 256
HID = 128
HEADS = 4
C1 = HID // HEADS
OUT = 2
NEG = 0.2
P = 128
NCORES = 8
N_ST_DATA = 784                   # data blocks (ceil(100000/128) = 782 -> 784 for /8)
T_PER_CORE = N_ST_DATA // NCORES + 1   # 99 (incl. all-dummy tail block)
N_ST = T_PER_CORE * NCORES        # 792
N_PAD = N_ST * P                  # 101376
ROWS_PER_CORE = T_PER_CORE * P    # 12672
DUMMY1 = N_PAD                    # table1 dummy row (extra row)
DUMMY2_LOCAL = (T_PER_CORE - 1) * P   # local row 12544 in every core's block
PAD_ALPHA = -1.0e4
D1 = HID + 2 * HEADS              # 136


def _host_prep(x, edge_index, W1, att_src1, att_dst1, b1, W2, att_src2, att_dst2, b2):
    src0 = np.asarray(edge_index[0], dtype=np.int64)
    dst0 = np.asarray(edge_index[1], dtype=np.int64)
    loops = np.arange(N, dtype=np.int64)
    src = np.concatenate([src0, loops])
    dst = np.concatenate([dst0, loops])

    deg = np.bincount(dst, minlength=N)
    order = np.argsort(deg, kind="stable")        # sorted position -> old id

    # sorted position p = s*128+d, block s = 8t+c  ->  fid = (c*T + t)*128 + d
    n_data = N_ST_DATA * P                        # 100352 sorted slots
    p_ar = np.arange(n_data, dtype=np.int64)
    s_ar = p_ar // P
    d_ar = p_ar % P
    t_ar = s_ar // NCORES
    c_ar = s_ar % NCORES
    fid_of_sorted = (c_ar * T_PER_CORE + t_ar) * P + d_ar   # [n_data]

    fid_of_old = np.empty(N, dtype=np.int64)
    fid_of_old[order] = fid_of_sorted[:N]
    old_of_fid = np.full(N_PAD, -1, dtype=np.int64)
    old_of_fid[fid_of_old] = np.arange(N)

    nsrc = fid_of_old[src]
    ndst = fid_of_old[dst]
    ndeg = np.zeros(N_PAD, dtype=np.int64)
    ndeg[fid_of_old] = deg

    K_ct = ndeg.reshape(NCORES, T_PER_CORE, P).max(-1)      # [8, 99]
    K_t = np.maximum(K_ct.max(0), 1)                        # [99]
    tot_slabs = int(K_t.sum())
    slab_off = np.concatenate([[0], np.cumsum(K_t)]).astype(np.int64)

    # slot index k of each edge within its dst; self-loop forced to k=0
    notself = (nsrc != ndst).astype(np.int8)
    key = np.lexsort((notself, ndst))
    ds_ = ndst[key]
    ss_ = nsrc[key]
    counts = np.bincount(ds_, minlength=N_PAD)
    run_start = np.zeros(N_PAD + 1, dtype=np.int64)
    run_start[1:] = np.cumsum(counts)
    kk = np.arange(ds_.size, dtype=np.int64) - run_start[ds_]

    blk = ds_ // P
    core = blk // T_PER_CORE
    t_idx = blk % T_PER_CORE
    d_rel = ds_ % P
    col = slab_off[t_idx] + kk

    idx1 = np.full((NCORES, P, tot_slabs), DUMMY1, dtype=np.int32)
    idx1[core, d_rel, col] = ss_.astype(np.int32)
    idx2 = idx1.copy()
    for c in range(NCORES):
        pad_val = c * ROWS_PER_CORE + DUMMY2_LOCAL
        m = idx2[c] == DUMMY1
        idx2[c][m] = pad_val

    # ---- phase-3 windowed dma_gather structures (currently unused) ----
    # table2big rows (256B bf16) indexed by fid; 4 windows of WS rows with
    # int16 window-local indices.  Self-loops handled separately (slab-0
    # style indirect gather on the compact table2), so windows hold only
    # non-self edges.  Window dummy = each window's resident core-tail
    # dummy row (alpha_src = -1e4 -> weight 0).
    WS = 32768
    NW = 4
    wdum = np.array([12544, 37888, 75904, 101248], dtype=np.int64)
    assert all(wdum // WS == np.arange(NW))
    ns_mask = ss_ != ds_
    dsn = ds_[ns_mask]
    ssn = ss_[ns_mask]
    wn = np.minimum(ssn // WS, NW - 1)
    blk_n = dsn // P
    core_n = blk_n // T_PER_CORE
    t_n = blk_n % T_PER_CORE
    p_n = dsn % P
    # per (core, t, p, w) counts and slot index within that bucket
    keyw = np.lexsort((ssn, wn, dsn))
    dsw = dsn[keyw]; ssw = ssn[keyw]; www = wn[keyw]
    bucket = dsw * NW + www
    cntw = np.bincount(bucket, minlength=N_PAD * NW)
    rs = np.zeros(N_PAD * NW + 1, dtype=np.int64)
    rs[1:] = np.cumsum(cntw)
    kw = np.arange(dsw.size, dtype=np.int64) - rs[bucket]
    cntw4 = cntw.reshape(N_PAD, NW)
    # K3_tw[t, w] = max over cores and dsts of row t (shared SPMD shape)
    K3_tw = cntw4.reshape(NCORES, T_PER_CORE, P, NW).max(2).max(0)  # [T, W]
    K3_tw = K3_tw[:T_PER_CORE - 1]          # data tiles only
    K3_t = K3_tw.sum(1)                      # [98]
    w_off = np.zeros((T_PER_CORE - 1, NW), dtype=np.int64)
    w_off[:, 1:] = np.cumsum(K3_tw, axis=1)[:, :-1]
    # flat int16 idx grid per core: for (t, w): [K3_tw, 128] slots
    # (k-major), value = window-local src fid (or window dummy)
    cols_t = (K3_tw * P // 16).sum(1)        # int16 cols per tile
    colbase = np.zeros(T_PER_CORE, dtype=np.int64)
    colbase[1:] = np.cumsum(cols_t)
    totcols = int(colbase[T_PER_CORE - 1])
    idx3 = np.empty((NCORES, 16, totcols), dtype=np.int16)
    for c in range(NCORES):
        # per-(t, w) slot grid, default = window-local dummy
        grid = {(t, w): np.full((int(K3_tw[t, w]), P), wdum[w] - w * WS,
                                dtype=np.int64)
                for t in range(T_PER_CORE - 1) for w in range(NW)}
        m_c = core_n[keyw] == c
        tt = t_n[keyw][m_c]; pp = p_n[keyw][m_c]
        wwc = www[m_c]; kkc = kw[m_c]; ssc = ssw[m_c]
        for t in range(T_PER_CORE - 1):
            mt = tt == t
            for w in range(NW):
                mw = mt & (wwc == w)
                g = grid[(t, w)]
                g[kkc[mw], pp[mw]] = ssc[mw] - w * WS
        pieces = []
        for t in range(T_PER_CORE - 1):
            for w in range(NW):
                g = grid[(t, w)]                     # [Kw, 128] k-major
                if g.size == 0:
                    continue
                fl = g.reshape(-1)                    # flat i = k*128+p
                pieces.append(fl.reshape(-1, 16).T)   # [16, n/16]
        idx3[c] = np.concatenate(pieces, axis=1).astype(np.int16)
    # self-loop (slab-0) indices into compact table2 per (core, tile)
    ixself = np.empty((NCORES, P, T_PER_CORE - 1), dtype=np.int32)
    for c in range(NCORES):
        for t in range(T_PER_CORE - 1):
            base = (c * T_PER_CORE + t) * P
            fids = np.arange(base, base + P)
            real = old_of_fid[fids] >= 0
            v = np.where(real, fids, c * ROWS_PER_CORE + DUMMY2_LOCAL)
            ixself[c, :, t] = v.astype(np.int32)

    # x tiles [792, 128, 2, 128] bf16: [s, p, i, n] = xp[s*128+n, i*128+p]
    # (partition-major contiguous so each block load is one flat 64KB DMA)
    import ml_dtypes
    xp = np.zeros((N_PAD, IN_DIM), dtype=np.float32)
    xp[fid_of_old] = np.asarray(x, dtype=np.float32)
    x_tiles = np.ascontiguousarray(
        xp.reshape(N_ST, P, 2, P).transpose(0, 3, 2, 1)).astype(ml_dtypes.bfloat16)

    A1s = np.zeros((HID, HEADS), dtype=np.float32)
    A1d = np.zeros((HID, HEADS), dtype=np.float32)
    for h in range(HEADS):
        A1s[h * C1:(h + 1) * C1, h] = np.asarray(att_src1, np.float32)[h]
        A1d[h * C1:(h + 1) * C1, h] = np.asarray(att_dst1, np.float32)[h]
    W1_ = np.asarray(W1, np.float32)
    W1aug = np.concatenate([W1_, W1_ @ A1s, W1_ @ A1d], axis=1)       # [256,136]
    W1aug_t = np.ascontiguousarray(
        W1aug.reshape(2, 128, D1).transpose(1, 0, 2)).astype(ml_dtypes.bfloat16)

    W2_ = np.asarray(W2, np.float32)
    a_s2 = np.asarray(att_src2, np.float32).reshape(OUT, 1)
    a_d2 = np.asarray(att_dst2, np.float32).reshape(OUT, 1)
    W2aug = np.concatenate([W2_, W2_ @ a_s2, W2_ @ a_d2], axis=1)     # [128,4]

    dummy1 = np.zeros((1, D1), dtype=np.float32)
    dummy1[0, HID:HID + HEADS] = PAD_ALPHA
    dummy2 = np.zeros((1, 4), dtype=np.float32)
    dummy2[0, 2] = PAD_ALPHA

    b1_b = np.tile(np.asarray(b1, np.float32)[None, :], (P, 1))
    b2_b = np.tile(np.asarray(b2, np.float32)[None, :], (P, 1))

    return dict(
        x_tiles=x_tiles, W1aug_t=W1aug_t, W2aug=W2aug,
        dummy1=dummy1, dummy2=dummy2, b1_b=b1_b, b2_b=b2_b,
        idx1=idx1, idx2=idx2, K_t=K_t, slab_off=slab_off,
        tot_slabs=tot_slabs, old_of_fid=old_of_fid,
        idx3=idx3, ixself=ixself, K3_tw=K3_tw, K3_t=K3_t, w_off=w_off,
        colbase=colbase, cols_t=cols_t, totcols=totcols,
    )


def _build_program(K_t, slab_off, tot_slabs, K3_tw, K3_t, w_off,
                   colbase, cols_t, totcols, dump_tables=False):
    import concourse.bass as bass
    import concourse.mybir as mybir
    from concourse.tile import TileContext
    from concourse.masks import make_identity

    f32 = mybir.dt.float32
    bf16 = mybir.dt.bfloat16
    i32 = mybir.dt.int32
    i16 = mybir.dt.int16
    AF = mybir.ActivationFunctionType
    OP = mybir.AluOpType

    nc = bass.Bass(target_bir_lowering=False)

    x_tiles = nc.dram_tensor("x_tiles", [N_ST, P, 2, P], bf16, kind="ExternalInput")
    w1aug = nc.dram_tensor("w1aug", [P, 2, D1], bf16, kind="ExternalInput")
    w2aug = nc.dram_tensor("w2aug", [HID, 4], f32, kind="ExternalInput")
    dummy1 = nc.dram_tensor("dummy1", [1, D1], f32, kind="ExternalInput")
    dummy2 = nc.dram_tensor("dummy2", [1, 4], f32, kind="ExternalInput")
    b1_b = nc.dram_tensor("b1_b", [P, HID], f32, kind="ExternalInput")
    b2_b = nc.dram_tensor("b2_b", [P, OUT], f32, kind="ExternalInput")
    idx1_in = nc.dram_tensor("idx1", [P, tot_slabs], i32, kind="ExternalInput")
    idx2_in = nc.dram_tensor("idx2", [P, tot_slabs], i32, kind="ExternalInput")
    out2 = nc.dram_tensor("out2", [ROWS_PER_CORE, OUT], f32, kind="ExternalOutput")
    if dump_tables:
        t1dump = nc.dram_tensor("t1dump", [N_PAD + 1, D1], f32, kind="ExternalOutput")
        t2dump = nc.dram_tensor("t2dump", [N_PAD, 4], f32, kind="ExternalOutput")

    with TileContext(nc) as tc:
        with tc.tile_pool(name="dram", bufs=1, space="DRAM") as dpool, \
             tc.tile_pool(name="const", bufs=1) as cpool, \
             tc.tile_pool(name="sb", bufs=3) as sb, \
             tc.tile_pool(name="gat", bufs=5) as gat, \
             tc.tile_pool(name="acc", bufs=2) as accp, \
             tc.tile_pool(name="ps", bufs=2, space="PSUM") as ps, \
             tc.tile_pool(name="p1", bufs=2) as p1pool:

            table1 = dpool.tile([N_PAD + 1, D1], f32)
            table2l = dpool.tile([ROWS_PER_CORE, 4], f32)
            table2 = dpool.tile([N_PAD, 4], f32, addr_space="Shared")

            # ---- constants ----
            w1_sb = cpool.tile([P, 2, D1], bf16)
            nc.sync.dma_start(out=w1_sb[:], in_=w1aug[:, :, :])
            w2_sb = cpool.tile([HID, 4], f32)
            nc.sync.dma_start(out=w2_sb[:], in_=w2aug[:, :])
            d1_sb = cpool.tile([1, D1], f32)
            nc.sync.dma_start(out=d1_sb[:], in_=dummy1[:, :])
            d2_sb = cpool.tile([1, 4], f32)
            nc.sync.dma_start(out=d2_sb[:], in_=dummy2[:, :])
            b1_sb = cpool.tile([P, HID], f32)
            nc.sync.dma_start(out=b1_sb[:], in_=b1_b[:, :])
            b2_sb = cpool.tile([P, OUT], f32)
            nc.sync.dma_start(out=b2_sb[:], in_=b2_b[:, :])
            ident = cpool.tile([P, P], f32)
            make_identity(nc, ident[:])

            nc.sync.dma_start(out=table1[N_PAD:N_PAD + 1, :], in_=d1_sb[:])
            nc.sync.dma_start(
                out=table2l[DUMMY2_LOCAL:DUMMY2_LOCAL + 1, :], in_=d2_sb[:])

            # ---- Phase 1: table1 (replicated on every core) ----
            # 8 tiles per DMA instruction (HWDGE issue cost ~0.6us dominates
            # phase 1 otherwise).
            SG = 8
            for s0 in range(0, N_ST, SG):
                xt = p1pool.tile([P, SG, 2, P], bf16, tag="xt")
                nc.sync.dma_start(
                    out=xt[:],
                    in_=x_tiles[s0:s0 + SG, :, :, :].rearrange(
                        "s p i n -> p s i n"))
                ht = p1pool.tile([P, SG, D1], f32, tag="ht")
                for j in range(SG):
                    hp = ps.tile([P, D1], f32, space="PSUM", tag="hp")
                    for i in range(2):
                        nc.tensor.matmul(out=hp[:], lhsT=xt[:, j, i, :],
                                         rhs=w1_sb[:, i, :],
                                         start=(i == 0), stop=(i == 1))
                    nc.vector.tensor_copy(ht[:, j, :], hp[:])
                nc.sync.dma_start(
                    out=table1[s0 * P:(s0 + SG) * P, :].rearrange(
                        "(s p) d -> p s d", s=SG),
                    in_=ht[:])

            # ---- preload the full idx arrays into SBUF once ----
            ix1_all = cpool.tile([P, tot_slabs], i32)
            nc.sync.dma_start(out=ix1_all[:], in_=idx1_in[:, :])
            ix2_all = cpool.tile([P, tot_slabs], i32)
            nc.sync.dma_start(out=ix2_all[:], in_=idx2_in[:, :])

            # ---- Phase 2: layer-1 aggregation -> table2_local ----
            for t in range(T_PER_CORE - 1):
                K = int(K_t[t])
                off = int(slab_off[t])
                gall = gat.tile([P, K, D1], f32, tag="gall")
                for k in range(K):
                    nc.gpsimd.indirect_dma_start(
                        out=gall[:, k, :], out_offset=None, in_=table1[:],
                        in_offset=bass.IndirectOffsetOnAxis(
                            ap=ix1_all[:, off + k:off + k + 1], axis=0))
                # e[p, h, k] = a_src(g_k) + a_dst(g_0)
                w_all = sb.tile([P, HEADS, K], f32, tag="w_all")
                nc.vector.tensor_tensor(
                    out=w_all[:],
                    in0=gall[:, :, HID:HID + HEADS].rearrange("p k h -> p h k"),
                    in1=gall[:, 0, HID + HEADS:HID + 2 * HEADS].to_broadcast(
                        [P, HEADS, K]),
                    op=OP.add)
                nc.scalar.activation(w_all[:], w_all[:], AF.Prelu, alpha=NEG)
                nc.scalar.activation(w_all[:], w_all[:], AF.Exp)
                den = sb.tile([P, HEADS], f32, tag="den")
                nc.vector.tensor_reduce(out=den[:], in_=w_all[:],
                                        axis=mybir.AxisListType.X, op=OP.add)
                rden = sb.tile([P, HEADS], f32, tag="rden")
                nc.vector.tensor_scalar(out=rden[:], in0=den[:], scalar1=1e-30,
                                        scalar2=None, op0=OP.add)
                nc.vector.reciprocal(rden[:], rden[:])
                u_all = accp.tile([P, K, HID], f32, tag="u_all")
                nc.vector.tensor_tensor(
                    out=u_all[:].rearrange("p k (h c) -> p k h c", h=HEADS),
                    in0=gall[:, :, 0:HID].rearrange("p k (h c) -> p k h c", h=HEADS),
                    in1=w_all[:].rearrange("p h k -> p k h").to_broadcast(
                        [P, K, HEADS, C1]),
                    op=OP.mult)
                acc = sb.tile([P, HID], f32, tag="acc")
                nc.vector.tensor_reduce(
                    out=acc[:], in_=u_all[:].rearrange("p k f -> p f k"),
                    axis=mybir.AxisListType.X, op=OP.add)
                h1 = sb.tile([P, HID], f32, tag="h1")
                nc.vector.tensor_tensor(
                    out=h1[:].rearrange("p (h c) -> p h c", h=HEADS),
                    in0=acc[:].rearrange("p (h c) -> p h c", h=HEADS),
                    in1=rden[:].to_broadcast([P, HEADS, C1]),
                    op=OP.mult)
                nc.vector.tensor_add(h1[:], h1[:], b1_sb[:])
                # elu
                m = sb.tile([P, HID], f32, tag="m")
                nc.vector.tensor_scalar(out=m[:], in0=h1[:], scalar1=0.0,
                                        scalar2=None, op0=OP.min)
                em = sb.tile([P, HID], f32, tag="em")
                nc.scalar.activation(em[:], m[:], AF.Exp)
                nc.vector.tensor_scalar(out=h1[:], in0=h1[:], scalar1=0.0,
                                        scalar2=None, op0=OP.max)
                nc.vector.tensor_add(h1[:], h1[:], em[:])
                nc.vector.tensor_scalar(out=h1[:], in0=h1[:], scalar1=1.0,
                                        scalar2=None, op0=OP.subtract)
                # table2 row block = h1 @ W2aug (via PE transpose)
                h1tp = ps.tile([P, P], f32, space="PSUM", tag="h1tp")
                nc.tensor.transpose(out=h1tp[:], in_=h1[:], identity=ident[:])
                h1t = sb.tile([P, P], f32, tag="h1t")
                nc.vector.tensor_copy(h1t[:], h1tp[:])
                t2p = ps.tile([P, 4], f32, space="PSUM", tag="t2p")
                nc.tensor.matmul(out=t2p[:], lhsT=h1t[:], rhs=w2_sb[:],
                                 start=True, stop=True)
                t2 = sb.tile([P, 4], f32, tag="t2")
                nc.vector.tensor_copy(t2[:], t2p[:])
                nc.sync.dma_start(out=table2l[t * P:(t + 1) * P, :], in_=t2[:])

            # ---- AllGather table2 ----
            nc.gpsimd.collective_compute(
                "AllGather", mybir.AluOpType.bypass,
                replica_groups=[list(range(NCORES))],
                ins=[table2l[:, :]], outs=[table2[:, :]])

            # ---- Phase 3: layer-2 aggregation -> out2 ----
            for t in range(T_PER_CORE - 1):
                K = int(K_t[t])
                off = int(slab_off[t])
                gall = gat.tile([P, K, 4], f32, tag="gall3")
                # slab 0 is the self-loop: dst's own row sits at the SAME
                # local address t*128+p in every core's table2l -> direct
                # HWDGE load instead of an indirect gather (saves ~1.4us of
                # serial gpsimd SWDGE time per tile). Pad dsts read their own
                # zero row (self-weight 1, den>=1); their output is discarded.
                nc.sync.dma_start(out=gall[:, 0, :],
                                  in_=table2l[t * P:(t + 1) * P, :])
                for k in range(1, K):
                    nc.gpsimd.indirect_dma_start(
                        out=gall[:, k, :], out_offset=None, in_=table2[:],
                        in_offset=bass.IndirectOffsetOnAxis(
                            ap=ix2_all[:, off + k:off + k + 1], axis=0))
                w_all = sb.tile([P, K], f32, tag="w_all3")
                nc.vector.tensor_tensor(
                    out=w_all[:],
                    in0=gall[:, :, 2:3].rearrange("p k o -> p (k o)"),
                    in1=gall[:, 0, 3:4].to_broadcast([P, K]),
                    op=OP.add)
                nc.scalar.activation(w_all[:], w_all[:], AF.Prelu, alpha=NEG)
                nc.scalar.activation(w_all[:], w_all[:], AF.Exp)
                den = sb.tile([P, 1], f32, tag="den3")
                nc.vector.tensor_reduce(out=den[:], in_=w_all[:],
                                        axis=mybir.AxisListType.X, op=OP.add)
                u_all = sb.tile([P, K, OUT], f32, tag="u_all3")
                nc.vector.tensor_tensor(
                    out=u_all[:],
                    in0=gall[:, :, 0:OUT],
                    in1=w_all[:].to_broadcast([P, K, OUT]),
                    op=OP.mult)
                acc = sb.tile([P, OUT], f32, tag="acc3")
                nc.vector.tensor_reduce(
                    out=acc[:], in_=u_all[:].rearrange("p k o -> p o k"),
                    axis=mybir.AxisListType.X, op=OP.add)
                rden = sb.tile([P, 1], f32, tag="rden3")
                nc.vector.tensor_scalar(out=rden[:], in0=den[:], scalar1=1e-30,
                                        scalar2=None, op0=OP.add)
                nc.vector.reciprocal(rden[:], rden[:])
                o2 = sb.tile([P, OUT], f32, tag="o2")
                nc.vector.tensor_tensor(out=o2[:], in0=acc[:],
                                        in1=rden[:].to_broadcast([P, OUT]),
                                        op=OP.mult)
                nc.vector.tensor_add(o2[:], o2[:], b2_sb[:])
                nc.sync.dma_start(out=out2[t * P:(t + 1) * P, :], in_=o2[:])

            if dump_tables:
                nc.sync.dma_start(out=t1dump[:, :], in_=table1[:])
                nc.sync.dma_start(out=t2dump[:, :], in_=table2[:])

    _split_waits(nc)
    return nc


def _split_waits(nc, max_waits=1):
    """This walrus build allows one sync-wait slot per instruction; hoist
    excess waits onto same-engine InstNoOp carriers inserted just before."""
    import concourse.mybir as mybir
    ctr = 0
    for fn in nc.m.functions:
        for bb in fn.blocks:
            out = []
            changed = False
            for inst in bb.instructions:
                si = inst.sync_info
                if si is not None and len(si.on_wait) > max_waits:
                    waits = list(si.on_wait)
                    extra, keep = waits[:-max_waits], waits[-max_waits:]
                    for i in range(0, len(extra), max_waits):
                        ctr += 1
                        nop = mybir.InstNoOp(name=f"waitfix-{ctr}", ins=[], outs=[])
                        nop.engine = inst.engine
                        nop.sync_info = mybir.SyncInfo(
                            on_wait=extra[i:i + max_waits], on_update=[])
                        out.append(nop)
                    si.on_wait = keep
                    inst.sync_info = si
                    changed = True
                out.append(inst)
            if changed:
                bb.instructions = out
    return ctr


def kernel(x, edge_index, W1, att_src1, att_dst1, b1, W2, att_src2, att_dst2, b2):
    from concourse.bass_utils import run_bass_kernel_spmd

    prep = _host_prep(x, edge_index, W1, att_src1, att_dst1, b1,
                      W2, att_src2, att_dst2, b2)
    nc = _build_program(prep["K_t"], prep["slab_off"], prep["tot_slabs"],
                        prep["K3_tw"], prep["K3_t"], prep["w_off"],
                        prep["colbase"], prep["cols_t"], prep["totcols"])

    in_maps = []
    for c in range(NCORES):
        in_maps.append({
            "x_tiles": prep["x_tiles"],
            "w1aug": prep["W1aug_t"],
            "w2aug": prep["W2aug"],
            "dummy1": prep["dummy1"],
            "dummy2": prep["dummy2"],
            "b1_b": prep["b1_b"],
            "b2_b": prep["b2_b"],
            "idx1": np.ascontiguousarray(prep["idx1"][c]),
            "idx2": np.ascontiguousarray(prep["idx2"][c]),
        })

    trace = bool(os.environ.get("GAT_TRACE"))
    if trace:
        _install_ntff_shim()
    r = run_bass_kernel_spmd(nc, in_maps, core_ids=list(range(NCORES)),
                             trace=trace)
    if trace and r.exec_time_ns:
        print(f"HW exec time: {r.exec_time_ns} ns")

    old_of_fid = prep["old_of_fid"]
    out = np.zeros((N, OUT), dtype=np.float32)
    for c in range(NCORES):
        fid0 = c * ROWS_PER_CORE
        olds = old_of_fid[fid0:fid0 + ROWS_PER_CORE]
        m = olds >= 0
        out[olds[m]] = r.results[c]["out2"][m]
    return out


def _install_ntff_shim():
    """The image's antenv lacks axon_hooks; recreate it so trace=True works."""
    import sys, types
    if "antenv.axon_hooks" in sys.modules:
        return
    sys.path.insert(0, "/root/.axon_site/trn_agent_boot")
    try:
        import trn_boot
        hook = trn_boot._ntff_profile_via_ctypes("/opt/axon/libaxon_pjrt.so")
    except Exception:
        hook = None
    mod = types.ModuleType("antenv.axon_hooks")
    mod.get_axon_ntff_profile_hook = lambda: hook
    mod.set_axon_ntff_profile_hook = lambda h: None
    sys.modules["antenv.axon_hooks"] = mod



# revision 54
# speedup vs baseline: 1.0521x; 1.0260x over previous
"""GAT (2-layer: 4 heads -> 1 head) on 8 trn2 NeuronCores.

Strategy (dst-partitioned slot design):
 - Relabel nodes by in-degree (ascending) into core-major blocks:
   final id = c*(T*128) + t*128 + d, where sorted 128-node block s = 8t + c.
   Each core owns T = 99 blocks (98 data + 1 all-dummy tail block whose slot
   12544 holds the table2 "dummy row" at the same local address on every
   core). Slab structure K_t is shared by all cores -> one SPMD program.
 - Per super-tile, edges live in "slabs": slab k holds edge k of every dst
   (one edge per partition). Self-loops pinned to slab 0, so the slab-0
   gather also delivers each dst's own table row (alpha_dst for free).
 - Phase 1 (replicated): table1[n] = x[n] @ [W1 | W1@A1s | W1@A1d] (136 f32).
 - Phase 2: per slab, [P,1] indirect-gather of table1[src]; per-partition
   softmax accumulation (exp without max-subtraction -- padded slots gather
   a dummy row with alpha_src=-1e4, so w = exp(leaky(-1e4+a_d)) == 0).
   Epilogue computes h1 then table2_local[t*128+d] = [h2 | a_s2 | a_d2].
 - AllGather table2_local (12672x4) -> table2_full (101376x4, Shared).
 - Phase 3: same slab loop over 16-byte table2 rows -> out2 [12672, 2].
"""
import os
import numpy as np

N = 100000
E = 1600000
IN_DIM = 256
HID = 128
HEADS = 4
C1 = HID // HEADS
OUT = 2
NEG = 0.2
P = 128
NCORES = 8
N_ST_DATA = 784                   # data blocks (ceil(100000/128) = 782 -> 784 for /8)
T_PER_CORE = N_ST_DATA // NCORES + 1   # 99 (incl. all-dummy tail block)
N_ST = T_PER_CORE * NCORES        # 792
N_PAD = N_ST * P                  # 101376
ROWS_PER_CORE = T_PER_CORE * P    # 12672
DUMMY1 = N_PAD                    # table1 dummy row (extra row)
DUMMY2_LOCAL = (T_PER_CORE - 1) * P   # local row 12544 in every core's block
PAD_ALPHA = -1.0e4
D1 = HID + 2 * HEADS              # 136


def _host_prep(x, edge_index, W1, att_src1, att_dst1, b1, W2, att_src2, att_dst2, b2):
    src0 = np.asarray(edge_index[0], dtype=np.int64)
    dst0 = np.asarray(edge_index[1], dtype=np.int64)
    loops = np.arange(N, dtype=np.int64)
    src = np.concatenate([src0, loops])
    dst = np.concatenate([dst0, loops])

    deg = np.bincount(dst, minlength=N)
    order = np.argsort(deg, kind="stable")        # sorted position -> old id

    # sorted position p = s*128+d, block s = 8t+c  ->  fid = (c*T + t)*128 + d
    n_data = N_ST_DATA * P                        # 100352 sorted slots
    p_ar = np.arange(n_data, dtype=np.int64)
    s_ar = p_ar // P
    d_ar = p_ar % P
    t_ar = s_ar // NCORES
    c_ar = s_ar % NCORES
    fid_of_sorted = (c_ar * T_PER_CORE + t_ar) * P + d_ar   # [n_data]

    fid_of_old = np.empty(N, dtype=np.int64)
    fid_of_old[order] = fid_of_sorted[:N]
    old_of_fid = np.full(N_PAD, -1, dtype=np.int64)
    old_of_fid[fid_of_old] = np.arange(N)

    nsrc = fid_of_old[src]
    ndst = fid_of_old[dst]
    ndeg = np.zeros(N_PAD, dtype=np.int64)
    ndeg[fid_of_old] = deg

    K_ct = ndeg.reshape(NCORES, T_PER_CORE, P).max(-1)      # [8, 99]
    K_t = np.maximum(K_ct.max(0), 1)                        # [99]
    tot_slabs = int(K_t.sum())
    slab_off = np.concatenate([[0], np.cumsum(K_t)]).astype(np.int64)

    # slot index k of each edge within its dst; self-loop forced to k=0
    notself = (nsrc != ndst).astype(np.int8)
    key = np.lexsort((notself, ndst))
    ds_ = ndst[key]
    ss_ = nsrc[key]
    counts = np.bincount(ds_, minlength=N_PAD)
    run_start = np.zeros(N_PAD + 1, dtype=np.int64)
    run_start[1:] = np.cumsum(counts)
    kk = np.arange(ds_.size, dtype=np.int64) - run_start[ds_]

    blk = ds_ // P
    core = blk // T_PER_CORE
    t_idx = blk % T_PER_CORE
    d_rel = ds_ % P
    col = slab_off[t_idx] + kk

    idx1 = np.full((NCORES, P, tot_slabs), DUMMY1, dtype=np.int32)
    idx1[core, d_rel, col] = ss_.astype(np.int32)
    idx2 = idx1.copy()
    for c in range(NCORES):
        pad_val = c * ROWS_PER_CORE + DUMMY2_LOCAL
        m = idx2[c] == DUMMY1
        idx2[c][m] = pad_val

    # ---- phase-3 windowed dma_gather structures (currently unused) ----
    # table2big rows (256B bf16) indexed by fid; 4 windows of WS rows with
    # int16 window-local indices.  Self-loops handled separately (slab-0
    # style indirect gather on the compact table2), so windows hold only
    # non-self edges.  Window dummy = each window's resident core-tail
    # dummy row (alpha_src = -1e4 -> weight 0).
    WS = 32768
    NW = 4
    wdum = np.array([12544, 37888, 75904, 101248], dtype=np.int64)
    assert all(wdum // WS == np.arange(NW))
    ns_mask = ss_ != ds_
    dsn = ds_[ns_mask]
    ssn = ss_[ns_mask]
    wn = np.minimum(ssn // WS, NW - 1)
    blk_n = dsn // P
    core_n = blk_n // T_PER_CORE
    t_n = blk_n % T_PER_CORE
    p_n = dsn % P
    # per (core, t, p, w) counts and slot index within that bucket
    keyw = np.lexsort((ssn, wn, dsn))
    dsw = dsn[keyw]; ssw = ssn[keyw]; www = wn[keyw]
    bucket = dsw * NW + www
    cntw = np.bincount(bucket, minlength=N_PAD * NW)
    rs = np.zeros(N_PAD * NW + 1, dtype=np.int64)
    rs[1:] = np.cumsum(cntw)
    kw = np.arange(dsw.size, dtype=np.int64) - rs[bucket]
    cntw4 = cntw.reshape(N_PAD, NW)
    # K3_tw[t, w] = max over cores and dsts of row t (shared SPMD shape)
    K3_tw = cntw4.reshape(NCORES, T_PER_CORE, P, NW).max(2).max(0)  # [T, W]
    K3_tw = K3_tw[:T_PER_CORE - 1]          # data tiles only
    K3_t = K3_tw.sum(1)                      # [98]
    w_off = np.zeros((T_PER_CORE - 1, NW), dtype=np.int64)
    w_off[:, 1:] = np.cumsum(K3_tw, axis=1)[:, :-1]
    # flat int16 idx grid per core: for (t, w): [K3_tw, 128] slots
    # (k-major), value = window-local src fid (or window dummy)
    cols_t = (K3_tw * P // 16).sum(1)        # int16 cols per tile
    colbase = np.zeros(T_PER_CORE, dtype=np.int64)
    colbase[1:] = np.cumsum(cols_t)
    totcols = int(colbase[T_PER_CORE - 1])
    idx3 = np.empty((NCORES, 16, totcols), dtype=np.int16)
    for c in range(NCORES):
        # per-(t, w) slot grid, default = window-local dummy
        grid = {(t, w): np.full((int(K3_tw[t, w]), P), wdum[w] - w * WS,
                                dtype=np.int64)
                for t in range(T_PER_CORE - 1) for w in range(NW)}
        m_c = core_n[keyw] == c
        tt = t_n[keyw][m_c]; pp = p_n[keyw][m_c]
        wwc = www[m_c]; kkc = kw[m_c]; ssc = ssw[m_c]
        for t in range(T_PER_CORE - 1):
            mt = tt == t
            for w in range(NW):
                mw = mt & (wwc == w)
                g = grid[(t, w)]
                g[kkc[mw], pp[mw]] = ssc[mw] - w * WS
        pieces = []
        for t in range(T_PER_CORE - 1):
            for w in range(NW):
                g = grid[(t, w)]                     # [Kw, 128] k-major
                if g.size == 0:
                    continue
                fl = g.reshape(-1)                    # flat i = k*128+p
                pieces.append(fl.reshape(-1, 16).T)   # [16, n/16]
        idx3[c] = np.concatenate(pieces, axis=1).astype(np.int16)
    # self-loop (slab-0) indices into compact table2 per (core, tile)
    ixself = np.empty((NCORES, P, T_PER_CORE - 1), dtype=np.int32)
    for c in range(NCORES):
        for t in range(T_PER_CORE - 1):
            base = (c * T_PER_CORE + t) * P
            fids = np.arange(base, base + P)
            real = old_of_fid[fids] >= 0
            v = np.where(real, fids, c * ROWS_PER_CORE + DUMMY2_LOCAL)
            ixself[c, :, t] = v.astype(np.int32)

    # x tiles [792, 128, 2, 128] bf16: [s, p, i, n] = xp[s*128+n, i*128+p]
    # (partition-major contiguous so each block load is one flat 64KB DMA)
    import ml_dtypes
    xp = np.zeros((N_PAD, IN_DIM), dtype=np.float32)
    xp[fid_of_old] = np.asarray(x, dtype=np.float32)
    x_tiles = np.ascontiguousarray(
        xp.reshape(N_ST, P, 2, P).transpose(0, 3, 2, 1)).astype(ml_dtypes.bfloat16)

    A1s = np.zeros((HID, HEADS), dtype=np.float32)
    A1d = np.zeros((HID, HEADS), dtype=np.float32)
    for h in range(HEADS):
        A1s[h * C1:(h + 1) * C1, h] = np.asarray(att_src1, np.float32)[h]
        A1d[h * C1:(h + 1) * C1, h] = np.asarray(att_dst1, np.float32)[h]
    W1_ = np.asarray(W1, np.float32)
    W1aug = np.concatenate([W1_, W1_ @ A1s, W1_ @ A1d], axis=1)       # [256,136]
    W1aug_t = np.ascontiguousarray(
        W1aug.reshape(2, 128, D1).transpose(1, 0, 2)).astype(ml_dtypes.bfloat16)

    W2_ = np.asarray(W2, np.float32)
    a_s2 = np.asarray(att_src2, np.float32).reshape(OUT, 1)
    a_d2 = np.asarray(att_dst2, np.float32).reshape(OUT, 1)
    W2aug = np.concatenate([W2_, W2_ @ a_s2, W2_ @ a_d2], axis=1)     # [128,4]

    dummy1 = np.zeros((1, D1), dtype=np.float32)
    dummy1[0, HID:HID + HEADS] = PAD_ALPHA
    dummy2 = np.zeros((1, 4), dtype=np.float32)
    dummy2[0, 2] = PAD_ALPHA

    b1_b = np.tile(np.asarray(b1, np.float32)[None, :], (P, 1))
    b2_b = np.tile(np.asarray(b2, np.float32)[None, :], (P, 1))

    # Per-core table1 row permutation: each core's OWN 99 blocks first, so
    # its own rows sit at local rows [t*128,(t+1)*128) -> phase-2 slab-0
    # (self-loop) becomes a direct DMA with a core-independent immediate.
    # table1 is per-core DRAM; only idx1 (per-core data) ties fid -> row.
    x_tiles_pc = []
    idx1_pc = []
    for c in range(NCORES):
        own = np.arange(c * T_PER_CORE, (c + 1) * T_PER_CORE)
        others = np.concatenate([np.arange(0, c * T_PER_CORE),
                                 np.arange((c + 1) * T_PER_CORE, N_ST)])
        order_c = np.concatenate([own, others])      # new s -> old block
        pos = np.empty(N_ST, dtype=np.int64)
        pos[order_c] = np.arange(N_ST)               # old block -> new block
        x_tiles_pc.append(np.ascontiguousarray(x_tiles[order_c]))
        v = idx1[c].copy()
        m = v != DUMMY1
        v[m] = (pos[v[m] // P] * P + v[m] % P).astype(np.int32)
        idx1_pc.append(v)

    return dict(
        x_tiles_pc=x_tiles_pc, W1aug_t=W1aug_t, W2aug=W2aug,
        dummy1=dummy1, dummy2=dummy2, b1_b=b1_b, b2_b=b2_b,
        idx1_pc=idx1_pc, idx2=idx2, K_t=K_t, slab_off=slab_off,
        tot_slabs=tot_slabs, old_of_fid=old_of_fid,
        idx3=idx3, ixself=ixself, K3_tw=K3_tw, K3_t=K3_t, w_off=w_off,
        colbase=colbase, cols_t=cols_t, totcols=totcols,
    )


def _build_program(K_t, slab_off, tot_slabs, K3_tw, K3_t, w_off,
                   colbase, cols_t, totcols, dump_tables=False):
    import concourse.bass as bass
    import concourse.mybir as mybir
    from concourse.tile import TileContext
    from concourse.masks import make_identity

    f32 = mybir.dt.float32
    bf16 = mybir.dt.bfloat16
    i32 = mybir.dt.int32
    i16 = mybir.dt.int16
    AF = mybir.ActivationFunctionType
    OP = mybir.AluOpType

    nc = bass.Bass(target_bir_lowering=False)

    x_tiles = nc.dram_tensor("x_tiles", [N_ST, P, 2, P], bf16, kind="ExternalInput")
    w1aug = nc.dram_tensor("w1aug", [P, 2, D1], bf16, kind="ExternalInput")
    w2aug = nc.dram_tensor("w2aug", [HID, 4], f32, kind="ExternalInput")
    dummy1 = nc.dram_tensor("dummy1", [1, D1], f32, kind="ExternalInput")
    dummy2 = nc.dram_tensor("dummy2", [1, 4], f32, kind="ExternalInput")
    b1_b = nc.dram_tensor("b1_b", [P, HID], f32, kind="ExternalInput")
    b2_b = nc.dram_tensor("b2_b", [P, OUT], f32, kind="ExternalInput")
    idx1_in = nc.dram_tensor("idx1", [P, tot_slabs], i32, kind="ExternalInput")
    idx2_in = nc.dram_tensor("idx2", [P, tot_slabs], i32, kind="ExternalInput")
    out2 = nc.dram_tensor("out2", [ROWS_PER_CORE, OUT], f32, kind="ExternalOutput")
    if dump_tables:
        t1dump = nc.dram_tensor("t1dump", [N_PAD + 1, D1], f32, kind="ExternalOutput")
        t2dump = nc.dram_tensor("t2dump", [N_PAD, 4], f32, kind="ExternalOutput")

    with TileContext(nc) as tc:
        with tc.tile_pool(name="dram", bufs=1, space="DRAM") as dpool, \
             tc.tile_pool(name="const", bufs=1) as cpool, \
             tc.tile_pool(name="sb", bufs=3) as sb, \
             tc.tile_pool(name="gat", bufs=5) as gat, \
             tc.tile_pool(name="acc", bufs=2) as accp, \
             tc.tile_pool(name="ps", bufs=2, space="PSUM") as ps, \
             tc.tile_pool(name="p1", bufs=2) as p1pool:

            table1 = dpool.tile([N_PAD + 1, D1], f32)
            table2l = dpool.tile([ROWS_PER_CORE, 4], f32)
            table2 = dpool.tile([N_PAD, 4], f32, addr_space="Shared")

            # ---- constants ----
            w1_sb = cpool.tile([P, 2, D1], bf16)
            nc.sync.dma_start(out=w1_sb[:], in_=w1aug[:, :, :])
            w2_sb = cpool.tile([HID, 4], f32)
            nc.sync.dma_start(out=w2_sb[:], in_=w2aug[:, :])
            d1_sb = cpool.tile([1, D1], f32)
            nc.sync.dma_start(out=d1_sb[:], in_=dummy1[:, :])
            d2_sb = cpool.tile([1, 4], f32)
            nc.sync.dma_start(out=d2_sb[:], in_=dummy2[:, :])
            b1_sb = cpool.tile([P, HID], f32)
            nc.sync.dma_start(out=b1_sb[:], in_=b1_b[:, :])
            b2_sb = cpool.tile([P, OUT], f32)
            nc.sync.dma_start(out=b2_sb[:], in_=b2_b[:, :])
            ident = cpool.tile([P, P], f32)
            make_identity(nc, ident[:])

            nc.sync.dma_start(out=table1[N_PAD:N_PAD + 1, :], in_=d1_sb[:])
            nc.sync.dma_start(
                out=table2l[DUMMY2_LOCAL:DUMMY2_LOCAL + 1, :], in_=d2_sb[:])

            # ---- Phase 1: table1 (replicated on every core) ----
            # 8 tiles per DMA instruction (HWDGE issue cost ~0.6us dominates
            # phase 1 otherwise).
            SG = 8
            for s0 in range(0, N_ST, SG):
                xt = p1pool.tile([P, SG, 2, P], bf16, tag="xt")
                nc.sync.dma_start(
                    out=xt[:],
                    in_=x_tiles[s0:s0 + SG, :, :, :].rearrange(
                        "s p i n -> p s i n"))
                ht = p1pool.tile([P, SG, D1], f32, tag="ht")
                for j in range(SG):
                    hp = ps.tile([P, D1], f32, space="PSUM", tag="hp")
                    for i in range(2):
                        nc.tensor.matmul(out=hp[:], lhsT=xt[:, j, i, :],
                                         rhs=w1_sb[:, i, :],
                                         start=(i == 0), stop=(i == 1))
                    nc.vector.tensor_copy(ht[:, j, :], hp[:])
                nc.sync.dma_start(
                    out=table1[s0 * P:(s0 + SG) * P, :].rearrange(
                        "(s p) d -> p s d", s=SG),
                    in_=ht[:])

            # ---- preload the full idx arrays into SBUF once ----
            ix1_all = cpool.tile([P, tot_slabs], i32)
            nc.sync.dma_start(out=ix1_all[:], in_=idx1_in[:, :])
            ix2_all = cpool.tile([P, tot_slabs], i32)
            nc.sync.dma_start(out=ix2_all[:], in_=idx2_in[:, :])

            # ---- Phase 2: layer-1 aggregation -> table2_local ----
            for t in range(T_PER_CORE - 1):
                K = int(K_t[t])
                off = int(slab_off[t])
                gall = gat.tile([P, K, D1], f32, tag="gall")
                # slab 0 = self-loop: own rows are at local rows t*128+p on
                # every core (per-core x_tiles/idx1 reordering) -> direct
                # load instead of an indirect gather.
                nc.sync.dma_start(out=gall[:, 0, :],
                                  in_=table1[t * P:(t + 1) * P, :])
                for k in range(1, K):
                    nc.gpsimd.indirect_dma_start(
                        out=gall[:, k, :], out_offset=None, in_=table1[:],
                        in_offset=bass.IndirectOffsetOnAxis(
                            ap=ix1_all[:, off + k:off + k + 1], axis=0))
                # e[p, h, k] = a_src(g_k) + a_dst(g_0)
                w_all = sb.tile([P, HEADS, K], f32, tag="w_all")
                nc.vector.tensor_tensor(
                    out=w_all[:],
                    in0=gall[:, :, HID:HID + HEADS].rearrange("p k h -> p h k"),
                    in1=gall[:, 0, HID + HEADS:HID + 2 * HEADS].to_broadcast(
                        [P, HEADS, K]),
                    op=OP.add)
                nc.scalar.activation(w_all[:], w_all[:], AF.Prelu, alpha=NEG)
                nc.scalar.activation(w_all[:], w_all[:], AF.Exp)
                den = sb.tile([P, HEADS], f32, tag="den")
                nc.vector.tensor_reduce(out=den[:], in_=w_all[:],
                                        axis=mybir.AxisListType.X, op=OP.add)
                rden = sb.tile([P, HEADS], f32, tag="rden")
                nc.vector.tensor_scalar(out=rden[:], in0=den[:], scalar1=1e-30,
                                        scalar2=None, op0=OP.add)
                nc.vector.reciprocal(rden[:], rden[:])
                u_all = accp.tile([P, K, HID], f32, tag="u_all")
                nc.vector.tensor_tensor(
                    out=u_all[:].rearrange("p k (h c) -> p k h c", h=HEADS),
                    in0=gall[:, :, 0:HID].rearrange("p k (h c) -> p k h c", h=HEADS),
                    in1=w_all[:].rearrange("p h k -> p k h").to_broadcast(
                        [P, K, HEADS, C1]),
                    op=OP.mult)
                acc = sb.tile([P, HID], f32, tag="acc")
                nc.vector.tensor_reduce(
                    out=acc[:], in_=u_all[:].rearrange("p k f -> p f k"),
                    axis=mybir.AxisListType.X, op=OP.add)
                h1 = sb.tile([P, HID], f32, tag="h1")
                nc.vector.tensor_tensor(
                    out=h1[:].rearrange("p (h c) -> p h c", h=HEADS),
                    in0=acc[:].rearrange("p (h c) -> p h c", h=HEADS),
                    in1=rden[:].to_broadcast([P, HEADS, C1]),
                    op=OP.mult)
                nc.vector.tensor_add(h1[:], h1[:], b1_sb[:])
                # elu
                m = sb.tile([P, HID], f32, tag="m")
                nc.vector.tensor_scalar(out=m[:], in0=h1[:], scalar1=0.0,
                                        scalar2=None, op0=OP.min)
                em = sb.tile([P, HID], f32, tag="em")
                nc.scalar.activation(em[:], m[:], AF.Exp)
                nc.vector.tensor_scalar(out=h1[:], in0=h1[:], scalar1=0.0,
                                        scalar2=None, op0=OP.max)
                nc.vector.tensor_add(h1[:], h1[:], em[:])
                nc.vector.tensor_scalar(out=h1[:], in0=h1[:], scalar1=1.0,
                                        scalar2=None, op0=OP.subtract)
                # table2 row block = h1 @ W2aug (via PE transpose)
                h1tp = ps.tile([P, P], f32, space="PSUM", tag="h1tp")
                nc.tensor.transpose(out=h1tp[:], in_=h1[:], identity=ident[:])
                h1t = sb.tile([P, P], f32, tag="h1t")
                nc.vector.tensor_copy(h1t[:], h1tp[:])
                t2p = ps.tile([P, 4], f32, space="PSUM", tag="t2p")
                nc.tensor.matmul(out=t2p[:], lhsT=h1t[:], rhs=w2_sb[:],
                                 start=True, stop=True)
                t2 = sb.tile([P, 4], f32, tag="t2")
                nc.vector.tensor_copy(t2[:], t2p[:])
                nc.sync.dma_start(out=table2l[t * P:(t + 1) * P, :], in_=t2[:])

            # ---- AllGather table2 ----
            nc.gpsimd.collective_compute(
                "AllGather", mybir.AluOpType.bypass,
                replica_groups=[list(range(NCORES))],
                ins=[table2l[:, :]], outs=[table2[:, :]])

            # ---- Phase 3: layer-2 aggregation -> out2 ----
            for t in range(T_PER_CORE - 1):
                K = int(K_t[t])
                off = int(slab_off[t])
                gall = gat.tile([P, K, 4], f32, tag="gall3")
                # slab 0 is the self-loop: dst's own row sits at the SAME
                # local address t*128+p in every core's table2l -> direct
                # HWDGE load instead of an indirect gather (saves ~1.4us of
                # serial gpsimd SWDGE time per tile). Pad dsts read their own
                # zero row (self-weight 1, den>=1); their output is discarded.
                nc.sync.dma_start(out=gall[:, 0, :],
                                  in_=table2l[t * P:(t + 1) * P, :])
                for k in range(1, K):
                    nc.gpsimd.indirect_dma_start(
                        out=gall[:, k, :], out_offset=None, in_=table2[:],
                        in_offset=bass.IndirectOffsetOnAxis(
                            ap=ix2_all[:, off + k:off + k + 1], axis=0))
                w_all = sb.tile([P, K], f32, tag="w_all3")
                nc.vector.tensor_tensor(
                    out=w_all[:],
                    in0=gall[:, :, 2:3].rearrange("p k o -> p (k o)"),
                    in1=gall[:, 0, 3:4].to_broadcast([P, K]),
                    op=OP.add)
                nc.scalar.activation(w_all[:], w_all[:], AF.Prelu, alpha=NEG)
                nc.scalar.activation(w_all[:], w_all[:], AF.Exp)
                den = sb.tile([P, 1], f32, tag="den3")
                nc.vector.tensor_reduce(out=den[:], in_=w_all[:],
                                        axis=mybir.AxisListType.X, op=OP.add)
                u_all = sb.tile([P, K, OUT], f32, tag="u_all3")
                nc.vector.tensor_tensor(
                    out=u_all[:],
                    in0=gall[:, :, 0:OUT],
                    in1=w_all[:].to_broadcast([P, K, OUT]),
                    op=OP.mult)
                acc = sb.tile([P, OUT], f32, tag="acc3")
                nc.vector.tensor_reduce(
                    out=acc[:], in_=u_all[:].rearrange("p k o -> p o k"),
                    axis=mybir.AxisListType.X, op=OP.add)
                rden = sb.tile([P, 1], f32, tag="rden3")
                nc.vector.tensor_scalar(out=rden[:], in0=den[:], scalar1=1e-30,
                                        scalar2=None, op0=OP.add)
                nc.vector.reciprocal(rden[:], rden[:])
                o2 = sb.tile([P, OUT], f32, tag="o2")
                nc.vector.tensor_tensor(out=o2[:], in0=acc[:],
                                        in1=rden[:].to_broadcast([P, OUT]),
                                        op=OP.mult)
                nc.vector.tensor_add(o2[:], o2[:], b2_sb[:])
                nc.sync.dma_start(out=out2[t * P:(t + 1) * P, :], in_=o2[:])

            if dump_tables:
                nc.sync.dma_start(out=t1dump[:, :], in_=table1[:])
                nc.sync.dma_start(out=t2dump[:, :], in_=table2[:])

    _split_waits(nc)
    return nc


def _split_waits(nc, max_waits=1):
    """This walrus build allows one sync-wait slot per instruction; hoist
    excess waits onto same-engine InstNoOp carriers inserted just before."""
    import concourse.mybir as mybir
    ctr = 0
    for fn in nc.m.functions:
        for bb in fn.blocks:
            out = []
            changed = False
            for inst in bb.instructions:
                si = inst.sync_info
                if si is not None and len(si.on_wait) > max_waits:
                    waits = list(si.on_wait)
                    extra, keep = waits[:-max_waits], waits[-max_waits:]
                    for i in range(0, len(extra), max_waits):
                        ctr += 1
                        nop = mybir.InstNoOp(name=f"waitfix-{ctr}", ins=[], outs=[])
                        nop.engine = inst.engine
                        nop.sync_info = mybir.SyncInfo(
                            on_wait=extra[i:i + max_waits], on_update=[])
                        out.append(nop)
                    si.on_wait = keep
                    inst.sync_info = si
                    changed = True
                out.append(inst)
            if changed:
                bb.instructions = out
    return ctr


def kernel(x, edge_index, W1, att_src1, att_dst1, b1, W2, att_src2, att_dst2, b2):
    from concourse.bass_utils import run_bass_kernel_spmd

    prep = _host_prep(x, edge_index, W1, att_src1, att_dst1, b1,
                      W2, att_src2, att_dst2, b2)
    nc = _build_program(prep["K_t"], prep["slab_off"], prep["tot_slabs"],
                        prep["K3_tw"], prep["K3_t"], prep["w_off"],
                        prep["colbase"], prep["cols_t"], prep["totcols"])

    in_maps = []
    for c in range(NCORES):
        in_maps.append({
            "x_tiles": prep["x_tiles_pc"][c],
            "w1aug": prep["W1aug_t"],
            "w2aug": prep["W2aug"],
            "dummy1": prep["dummy1"],
            "dummy2": prep["dummy2"],
            "b1_b": prep["b1_b"],
            "b2_b": prep["b2_b"],
            "idx1": np.ascontiguousarray(prep["idx1_pc"][c]),
            "idx2": np.ascontiguousarray(prep["idx2"][c]),
        })

    trace = bool(os.environ.get("GAT_TRACE"))
    if trace:
        _install_ntff_shim()
    r = run_bass_kernel_spmd(nc, in_maps, core_ids=list(range(NCORES)),
                             trace=trace)
    if trace and r.exec_time_ns:
        print(f"HW exec time: {r.exec_time_ns} ns")

    old_of_fid = prep["old_of_fid"]
    out = np.zeros((N, OUT), dtype=np.float32)
    for c in range(NCORES):
        fid0 = c * ROWS_PER_CORE
        olds = old_of_fid[fid0:fid0 + ROWS_PER_CORE]
        m = olds >= 0
        out[olds[m]] = r.results[c]["out2"][m]
    return out


def _install_ntff_shim():
    """The image's antenv lacks axon_hooks; recreate it so trace=True works."""
    import sys, types
    if "antenv.axon_hooks" in sys.modules:
        return
    sys.path.insert(0, "/root/.axon_site/trn_agent_boot")
    try:
        import trn_boot
        hook = trn_boot._ntff_profile_via_ctypes("/opt/axon/libaxon_pjrt.so")
    except Exception:
        hook = None
    mod = types.ModuleType("antenv.axon_hooks")
    mod.get_axon_ntff_profile_hook = lambda: hook
    mod.set_axon_ntff_profile_hook = lambda h: None
    sys.modules["antenv.axon_hooks"] = mod



# revision 55
# speedup vs baseline: 1.0591x; 1.0067x over previous
"""GAT (2-layer: 4 heads -> 1 head) on 8 trn2 NeuronCores.

Strategy (dst-partitioned slot design):
 - Relabel nodes by in-degree (ascending) into core-major blocks:
   final id = c*(T*128) + t*128 + d, where sorted 128-node block s = 8t + c.
   Each core owns T = 99 blocks (98 data + 1 all-dummy tail block whose slot
   12544 holds the table2 "dummy row" at the same local address on every
   core). Slab structure K_t is shared by all cores -> one SPMD program.
 - Per super-tile, edges live in "slabs": slab k holds edge k of every dst
   (one edge per partition). Self-loops pinned to slab 0, so the slab-0
   gather also delivers each dst's own table row (alpha_dst for free).
 - Phase 1 (replicated): table1[n] = x[n] @ [W1 | W1@A1s | W1@A1d] (136 f32).
 - Phase 2: per slab, [P,1] indirect-gather of table1[src]; per-partition
   softmax accumulation (exp without max-subtraction -- padded slots gather
   a dummy row with alpha_src=-1e4, so w = exp(leaky(-1e4+a_d)) == 0).
   Epilogue computes h1 then table2_local[t*128+d] = [h2 | a_s2 | a_d2].
 - AllGather table2_local (12672x4) -> table2_full (101376x4, Shared).
 - Phase 3: same slab loop over 16-byte table2 rows -> out2 [12672, 2].
"""
import os
import numpy as np

N = 100000
E = 1600000
IN_DIM = 256
HID = 128
HEADS = 4
C1 = HID // HEADS
OUT = 2
NEG = 0.2
P = 128
NCORES = 8
N_ST_DATA = 784                   # data blocks (ceil(100000/128) = 782 -> 784 for /8)
T_PER_CORE = N_ST_DATA // NCORES + 1   # 99 (incl. all-dummy tail block)
N_ST = T_PER_CORE * NCORES        # 792
N_PAD = N_ST * P                  # 101376
ROWS_PER_CORE = T_PER_CORE * P    # 12672
DUMMY1 = N_PAD                    # table1 dummy row (extra row)
DUMMY2_LOCAL = (T_PER_CORE - 1) * P   # local row 12544 in every core's block
PAD_ALPHA = -1.0e4
D1 = HID + 2 * HEADS              # 136


def _host_prep(x, edge_index, W1, att_src1, att_dst1, b1, W2, att_src2, att_dst2, b2):
    src0 = np.asarray(edge_index[0], dtype=np.int64)
    dst0 = np.asarray(edge_index[1], dtype=np.int64)
    loops = np.arange(N, dtype=np.int64)
    src = np.concatenate([src0, loops])
    dst = np.concatenate([dst0, loops])

    deg = np.bincount(dst, minlength=N)
    order = np.argsort(deg, kind="stable")        # sorted position -> old id

    # sorted position p = s*128+d, block s = 8t+c  ->  fid = (c*T + t)*128 + d
    n_data = N_ST_DATA * P                        # 100352 sorted slots
    p_ar = np.arange(n_data, dtype=np.int64)
    s_ar = p_ar // P
    d_ar = p_ar % P
    t_ar = s_ar // NCORES
    c_ar = s_ar % NCORES
    fid_of_sorted = (c_ar * T_PER_CORE + t_ar) * P + d_ar   # [n_data]

    fid_of_old = np.empty(N, dtype=np.int64)
    fid_of_old[order] = fid_of_sorted[:N]
    old_of_fid = np.full(N_PAD, -1, dtype=np.int64)
    old_of_fid[fid_of_old] = np.arange(N)

    nsrc = fid_of_old[src]
    ndst = fid_of_old[dst]
    ndeg = np.zeros(N_PAD, dtype=np.int64)
    ndeg[fid_of_old] = deg

    K_ct = ndeg.reshape(NCORES, T_PER_CORE, P).max(-1)      # [8, 99]
    K_t = np.maximum(K_ct.max(0), 1)                        # [99]
    tot_slabs = int(K_t.sum())
    slab_off = np.concatenate([[0], np.cumsum(K_t)]).astype(np.int64)

    # slot index k of each edge within its dst; self-loop forced to k=0
    notself = (nsrc != ndst).astype(np.int8)
    key = np.lexsort((notself, ndst))
    ds_ = ndst[key]
    ss_ = nsrc[key]
    counts = np.bincount(ds_, minlength=N_PAD)
    run_start = np.zeros(N_PAD + 1, dtype=np.int64)
    run_start[1:] = np.cumsum(counts)
    kk = np.arange(ds_.size, dtype=np.int64) - run_start[ds_]

    blk = ds_ // P
    core = blk // T_PER_CORE
    t_idx = blk % T_PER_CORE
    d_rel = ds_ % P
    col = slab_off[t_idx] + kk

    idx1 = np.full((NCORES, P, tot_slabs), DUMMY1, dtype=np.int32)
    idx1[core, d_rel, col] = ss_.astype(np.int32)
    idx2 = idx1.copy()
    for c in range(NCORES):
        pad_val = c * ROWS_PER_CORE + DUMMY2_LOCAL
        m = idx2[c] == DUMMY1
        idx2[c][m] = pad_val

    # ---- phase-3 windowed dma_gather structures (currently unused) ----
    # table2big rows (256B bf16) indexed by fid; 4 windows of WS rows with
    # int16 window-local indices.  Self-loops handled separately (slab-0
    # style indirect gather on the compact table2), so windows hold only
    # non-self edges.  Window dummy = each window's resident core-tail
    # dummy row (alpha_src = -1e4 -> weight 0).
    WS = 32768
    NW = 4
    wdum = np.array([12544, 37888, 75904, 101248], dtype=np.int64)
    assert all(wdum // WS == np.arange(NW))
    ns_mask = ss_ != ds_
    dsn = ds_[ns_mask]
    ssn = ss_[ns_mask]
    wn = np.minimum(ssn // WS, NW - 1)
    blk_n = dsn // P
    core_n = blk_n // T_PER_CORE
    t_n = blk_n % T_PER_CORE
    p_n = dsn % P
    # per (core, t, p, w) counts and slot index within that bucket
    keyw = np.lexsort((ssn, wn, dsn))
    dsw = dsn[keyw]; ssw = ssn[keyw]; www = wn[keyw]
    bucket = dsw * NW + www
    cntw = np.bincount(bucket, minlength=N_PAD * NW)
    rs = np.zeros(N_PAD * NW + 1, dtype=np.int64)
    rs[1:] = np.cumsum(cntw)
    kw = np.arange(dsw.size, dtype=np.int64) - rs[bucket]
    cntw4 = cntw.reshape(N_PAD, NW)
    # K3_tw[t, w] = max over cores and dsts of row t (shared SPMD shape)
    K3_tw = cntw4.reshape(NCORES, T_PER_CORE, P, NW).max(2).max(0)  # [T, W]
    K3_tw = K3_tw[:T_PER_CORE - 1]          # data tiles only
    K3_t = K3_tw.sum(1)                      # [98]
    w_off = np.zeros((T_PER_CORE - 1, NW), dtype=np.int64)
    w_off[:, 1:] = np.cumsum(K3_tw, axis=1)[:, :-1]
    # flat int16 idx grid per core: for (t, w): [K3_tw, 128] slots
    # (k-major), value = window-local src fid (or window dummy)
    cols_t = (K3_tw * P // 16).sum(1)        # int16 cols per tile
    colbase = np.zeros(T_PER_CORE, dtype=np.int64)
    colbase[1:] = np.cumsum(cols_t)
    totcols = int(colbase[T_PER_CORE - 1])
    idx3 = np.empty((NCORES, 16, totcols), dtype=np.int16)
    for c in range(NCORES):
        # per-(t, w) slot grid, default = window-local dummy
        grid = {(t, w): np.full((int(K3_tw[t, w]), P), wdum[w] - w * WS,
                                dtype=np.int64)
                for t in range(T_PER_CORE - 1) for w in range(NW)}
        m_c = core_n[keyw] == c
        tt = t_n[keyw][m_c]; pp = p_n[keyw][m_c]
        wwc = www[m_c]; kkc = kw[m_c]; ssc = ssw[m_c]
        for t in range(T_PER_CORE - 1):
            mt = tt == t
            for w in range(NW):
                mw = mt & (wwc == w)
                g = grid[(t, w)]
                g[kkc[mw], pp[mw]] = ssc[mw] - w * WS
        pieces = []
        for t in range(T_PER_CORE - 1):
            for w in range(NW):
                g = grid[(t, w)]                     # [Kw, 128] k-major
                if g.size == 0:
                    continue
                fl = g.reshape(-1)                    # flat i = k*128+p
                pieces.append(fl.reshape(-1, 16).T)   # [16, n/16]
        idx3[c] = np.concatenate(pieces, axis=1).astype(np.int16)
    # self-loop (slab-0) indices into compact table2 per (core, tile)
    ixself = np.empty((NCORES, P, T_PER_CORE - 1), dtype=np.int32)
    for c in range(NCORES):
        for t in range(T_PER_CORE - 1):
            base = (c * T_PER_CORE + t) * P
            fids = np.arange(base, base + P)
            real = old_of_fid[fids] >= 0
            v = np.where(real, fids, c * ROWS_PER_CORE + DUMMY2_LOCAL)
            ixself[c, :, t] = v.astype(np.int32)

    # x tiles [792, 128, 2, 128] bf16: [s, p, i, n] = xp[s*128+n, i*128+p]
    # (partition-major contiguous so each block load is one flat 64KB DMA)
    import ml_dtypes
    xp = np.zeros((N_PAD, IN_DIM), dtype=np.float32)
    xp[fid_of_old] = np.asarray(x, dtype=np.float32)
    x_tiles = np.ascontiguousarray(
        xp.reshape(N_ST, P, 2, P).transpose(0, 3, 2, 1)).astype(ml_dtypes.bfloat16)

    A1s = np.zeros((HID, HEADS), dtype=np.float32)
    A1d = np.zeros((HID, HEADS), dtype=np.float32)
    for h in range(HEADS):
        A1s[h * C1:(h + 1) * C1, h] = np.asarray(att_src1, np.float32)[h]
        A1d[h * C1:(h + 1) * C1, h] = np.asarray(att_dst1, np.float32)[h]
    W1_ = np.asarray(W1, np.float32)
    W1aug = np.concatenate([W1_, W1_ @ A1s, W1_ @ A1d], axis=1)       # [256,136]
    W1aug_t = np.ascontiguousarray(
        W1aug.reshape(2, 128, D1).transpose(1, 0, 2)).astype(ml_dtypes.bfloat16)

    W2_ = np.asarray(W2, np.float32)
    a_s2 = np.asarray(att_src2, np.float32).reshape(OUT, 1)
    a_d2 = np.asarray(att_dst2, np.float32).reshape(OUT, 1)
    W2aug = np.concatenate([W2_, W2_ @ a_s2, W2_ @ a_d2], axis=1)     # [128,4]

    dummy1 = np.zeros((1, D1), dtype=np.float32)
    dummy1[0, HID:HID + HEADS] = PAD_ALPHA
    dummy2 = np.zeros((1, 4), dtype=np.float32)
    dummy2[0, 2] = PAD_ALPHA

    b1_b = np.tile(np.asarray(b1, np.float32)[None, :], (P, 1))
    b2_b = np.tile(np.asarray(b2, np.float32)[None, :], (P, 1))

    # Per-core table1 row permutation: each core's OWN 99 blocks first, so
    # its own rows sit at local rows [t*128,(t+1)*128) -> phase-2 slab-0
    # (self-loop) becomes a direct DMA with a core-independent immediate.
    # table1 is per-core DRAM; only idx1 (per-core data) ties fid -> row.
    x_tiles_pc = []
    idx1_pc = []
    for c in range(NCORES):
        own = np.arange(c * T_PER_CORE, (c + 1) * T_PER_CORE)
        others = np.concatenate([np.arange(0, c * T_PER_CORE),
                                 np.arange((c + 1) * T_PER_CORE, N_ST)])
        order_c = np.concatenate([own, others])      # new s -> old block
        pos = np.empty(N_ST, dtype=np.int64)
        pos[order_c] = np.arange(N_ST)               # old block -> new block
        x_tiles_pc.append(np.ascontiguousarray(x_tiles[order_c]))
        v = idx1[c].copy()
        m = v != DUMMY1
        v[m] = (pos[v[m] // P] * P + v[m] % P).astype(np.int32)
        idx1_pc.append(v)

    return dict(
        x_tiles_pc=x_tiles_pc, W1aug_t=W1aug_t, W2aug=W2aug,
        dummy1=dummy1, dummy2=dummy2, b1_b=b1_b, b2_b=b2_b,
        idx1_pc=idx1_pc, idx2=idx2, K_t=K_t, slab_off=slab_off,
        tot_slabs=tot_slabs, old_of_fid=old_of_fid,
        idx3=idx3, ixself=ixself, K3_tw=K3_tw, K3_t=K3_t, w_off=w_off,
        colbase=colbase, cols_t=cols_t, totcols=totcols,
    )


def _build_program(K_t, slab_off, tot_slabs, K3_tw, K3_t, w_off,
                   colbase, cols_t, totcols, dump_tables=False):
    import concourse.bass as bass
    import concourse.mybir as mybir
    from concourse.tile import TileContext
    from concourse.masks import make_identity

    f32 = mybir.dt.float32
    bf16 = mybir.dt.bfloat16
    i32 = mybir.dt.int32
    i16 = mybir.dt.int16
    AF = mybir.ActivationFunctionType
    OP = mybir.AluOpType

    nc = bass.Bass(target_bir_lowering=False)

    x_tiles = nc.dram_tensor("x_tiles", [N_ST, P, 2, P], bf16, kind="ExternalInput")
    w1aug = nc.dram_tensor("w1aug", [P, 2, D1], bf16, kind="ExternalInput")
    w2aug = nc.dram_tensor("w2aug", [HID, 4], f32, kind="ExternalInput")
    dummy1 = nc.dram_tensor("dummy1", [1, D1], f32, kind="ExternalInput")
    dummy2 = nc.dram_tensor("dummy2", [1, 4], f32, kind="ExternalInput")
    b1_b = nc.dram_tensor("b1_b", [P, HID], f32, kind="ExternalInput")
    b2_b = nc.dram_tensor("b2_b", [P, OUT], f32, kind="ExternalInput")
    idx1_in = nc.dram_tensor("idx1", [P, tot_slabs], i32, kind="ExternalInput")
    idx2_in = nc.dram_tensor("idx2", [P, tot_slabs], i32, kind="ExternalInput")
    out2 = nc.dram_tensor("out2", [ROWS_PER_CORE, OUT], f32, kind="ExternalOutput")
    if dump_tables:
        t1dump = nc.dram_tensor("t1dump", [N_PAD + 1, D1], f32, kind="ExternalOutput")
        t2dump = nc.dram_tensor("t2dump", [N_PAD, 4], f32, kind="ExternalOutput")

    with TileContext(nc) as tc:
        with tc.tile_pool(name="dram", bufs=1, space="DRAM") as dpool, \
             tc.tile_pool(name="const", bufs=1) as cpool, \
             tc.tile_pool(name="sb", bufs=3) as sb, \
             tc.tile_pool(name="gat", bufs=5) as gat, \
             tc.tile_pool(name="acc", bufs=2) as accp, \
             tc.tile_pool(name="ps", bufs=2, space="PSUM") as ps, \
             tc.tile_pool(name="p1", bufs=2) as p1pool:

            table1 = dpool.tile([N_PAD + 1, D1], f32)
            table2l = dpool.tile([ROWS_PER_CORE, 4], f32)
            table2 = dpool.tile([N_PAD, 4], f32, addr_space="Shared")

            # ---- constants ----
            w1_sb = cpool.tile([P, 2, D1], bf16)
            nc.sync.dma_start(out=w1_sb[:], in_=w1aug[:, :, :])
            w2_sb = cpool.tile([HID, 4], f32)
            nc.sync.dma_start(out=w2_sb[:], in_=w2aug[:, :])
            d1_sb = cpool.tile([1, D1], f32)
            nc.sync.dma_start(out=d1_sb[:], in_=dummy1[:, :])
            d2_sb = cpool.tile([1, 4], f32)
            nc.sync.dma_start(out=d2_sb[:], in_=dummy2[:, :])
            b1_sb = cpool.tile([P, HID], f32)
            nc.sync.dma_start(out=b1_sb[:], in_=b1_b[:, :])
            b2_sb = cpool.tile([P, OUT], f32)
            nc.sync.dma_start(out=b2_sb[:], in_=b2_b[:, :])
            ident = cpool.tile([P, P], f32)
            make_identity(nc, ident[:])

            nc.sync.dma_start(out=table1[N_PAD:N_PAD + 1, :], in_=d1_sb[:])
            nc.sync.dma_start(
                out=table2l[DUMMY2_LOCAL:DUMMY2_LOCAL + 1, :], in_=d2_sb[:])

            # ---- Phase 1: table1 (replicated on every core) ----
            # 8 tiles per DMA instruction (HWDGE issue cost ~0.6us dominates
            # phase 1 otherwise).
            SG = 8
            for s0 in range(0, N_ST, SG):
                xt = p1pool.tile([P, SG, 2, P], bf16, tag="xt")
                nc.sync.dma_start(
                    out=xt[:],
                    in_=x_tiles[s0:s0 + SG, :, :, :].rearrange(
                        "s p i n -> p s i n"))
                ht = p1pool.tile([P, SG, D1], f32, tag="ht")
                for j in range(SG):
                    hp = ps.tile([P, D1], f32, space="PSUM", tag="hp")
                    for i in range(2):
                        nc.tensor.matmul(out=hp[:], lhsT=xt[:, j, i, :],
                                         rhs=w1_sb[:, i, :],
                                         start=(i == 0), stop=(i == 1))
                    nc.vector.tensor_copy(ht[:, j, :], hp[:])
                nc.sync.dma_start(
                    out=table1[s0 * P:(s0 + SG) * P, :].rearrange(
                        "(s p) d -> p s d", s=SG),
                    in_=ht[:])

            # ---- preload the full idx arrays into SBUF once ----
            ix1_all = cpool.tile([P, tot_slabs], i32)
            nc.sync.dma_start(out=ix1_all[:], in_=idx1_in[:, :])
            ix2_all = cpool.tile([P, tot_slabs], i32)
            nc.sync.dma_start(out=ix2_all[:], in_=idx2_in[:, :])

            # ---- Phase 2: layer-1 aggregation -> table2_local ----
            # reversed: highest-degree tiles first, so the serial tail
            # before the AllGather ends on the smallest tile
            for t in reversed(range(T_PER_CORE - 1)):
                K = int(K_t[t])
                off = int(slab_off[t])
                gall = gat.tile([P, K, D1], f32, tag="gall")
                # slab 0 = self-loop: own rows are at local rows t*128+p on
                # every core (per-core x_tiles/idx1 reordering) -> direct
                # load instead of an indirect gather.
                nc.sync.dma_start(out=gall[:, 0, :],
                                  in_=table1[t * P:(t + 1) * P, :])
                for k in range(1, K):
                    nc.gpsimd.indirect_dma_start(
                        out=gall[:, k, :], out_offset=None, in_=table1[:],
                        in_offset=bass.IndirectOffsetOnAxis(
                            ap=ix1_all[:, off + k:off + k + 1], axis=0))
                # e[p, h, k] = a_src(g_k) + a_dst(g_0)
                w_all = sb.tile([P, HEADS, K], f32, tag="w_all")
                nc.vector.tensor_tensor(
                    out=w_all[:],
                    in0=gall[:, :, HID:HID + HEADS].rearrange("p k h -> p h k"),
                    in1=gall[:, 0, HID + HEADS:HID + 2 * HEADS].to_broadcast(
                        [P, HEADS, K]),
                    op=OP.add)
                nc.scalar.activation(w_all[:], w_all[:], AF.Prelu, alpha=NEG)
                nc.scalar.activation(w_all[:], w_all[:], AF.Exp)
                den = sb.tile([P, HEADS], f32, tag="den")
                nc.vector.tensor_reduce(out=den[:], in_=w_all[:],
                                        axis=mybir.AxisListType.X, op=OP.add)
                rden = sb.tile([P, HEADS], f32, tag="rden")
                nc.vector.tensor_scalar(out=rden[:], in0=den[:], scalar1=1e-30,
                                        scalar2=None, op0=OP.add)
                nc.vector.reciprocal(rden[:], rden[:])
                u_all = accp.tile([P, K, HID], f32, tag="u_all")
                nc.vector.tensor_tensor(
                    out=u_all[:].rearrange("p k (h c) -> p k h c", h=HEADS),
                    in0=gall[:, :, 0:HID].rearrange("p k (h c) -> p k h c", h=HEADS),
                    in1=w_all[:].rearrange("p h k -> p k h").to_broadcast(
                        [P, K, HEADS, C1]),
                    op=OP.mult)
                acc = sb.tile([P, HID], f32, tag="acc")
                nc.vector.tensor_reduce(
                    out=acc[:], in_=u_all[:].rearrange("p k f -> p f k"),
                    axis=mybir.AxisListType.X, op=OP.add)
                h1 = sb.tile([P, HID], f32, tag="h1")
                nc.vector.tensor_tensor(
                    out=h1[:].rearrange("p (h c) -> p h c", h=HEADS),
                    in0=acc[:].rearrange("p (h c) -> p h c", h=HEADS),
                    in1=rden[:].to_broadcast([P, HEADS, C1]),
                    op=OP.mult)
                nc.vector.tensor_add(h1[:], h1[:], b1_sb[:])
                # elu
                m = sb.tile([P, HID], f32, tag="m")
                nc.vector.tensor_scalar(out=m[:], in0=h1[:], scalar1=0.0,
                                        scalar2=None, op0=OP.min)
                em = sb.tile([P, HID], f32, tag="em")
                nc.scalar.activation(em[:], m[:], AF.Exp)
                nc.vector.tensor_scalar(out=h1[:], in0=h1[:], scalar1=0.0,
                                        scalar2=None, op0=OP.max)
                nc.vector.tensor_add(h1[:], h1[:], em[:])
                nc.vector.tensor_scalar(out=h1[:], in0=h1[:], scalar1=1.0,
                                        scalar2=None, op0=OP.subtract)
                # table2 row block = h1 @ W2aug (via PE transpose)
                h1tp = ps.tile([P, P], f32, space="PSUM", tag="h1tp")
                nc.tensor.transpose(out=h1tp[:], in_=h1[:], identity=ident[:])
                h1t = sb.tile([P, P], f32, tag="h1t")
                nc.vector.tensor_copy(h1t[:], h1tp[:])
                t2p = ps.tile([P, 4], f32, space="PSUM", tag="t2p")
                nc.tensor.matmul(out=t2p[:], lhsT=h1t[:], rhs=w2_sb[:],
                                 start=True, stop=True)
                t2 = sb.tile([P, 4], f32, tag="t2")
                nc.vector.tensor_copy(t2[:], t2p[:])
                nc.sync.dma_start(out=table2l[t * P:(t + 1) * P, :], in_=t2[:])

            # ---- AllGather table2 ----
            nc.gpsimd.collective_compute(
                "AllGather", mybir.AluOpType.bypass,
                replica_groups=[list(range(NCORES))],
                ins=[table2l[:, :]], outs=[table2[:, :]])

            # ---- Phase 3: layer-2 aggregation -> out2 ----
            # reversed: kernel ends on the smallest tile's store
            for t in reversed(range(T_PER_CORE - 1)):
                K = int(K_t[t])
                off = int(slab_off[t])
                gall = gat.tile([P, K, 4], f32, tag="gall3")
                # slab 0 is the self-loop: dst's own row sits at the SAME
                # local address t*128+p in every core's table2l -> direct
                # HWDGE load instead of an indirect gather (saves ~1.4us of
                # serial gpsimd SWDGE time per tile). Pad dsts read their own
                # zero row (self-weight 1, den>=1); their output is discarded.
                nc.sync.dma_start(out=gall[:, 0, :],
                                  in_=table2l[t * P:(t + 1) * P, :])
                for k in range(1, K):
                    nc.gpsimd.indirect_dma_start(
                        out=gall[:, k, :], out_offset=None, in_=table2[:],
                        in_offset=bass.IndirectOffsetOnAxis(
                            ap=ix2_all[:, off + k:off + k + 1], axis=0))
                w_all = sb.tile([P, K], f32, tag="w_all3")
                nc.vector.tensor_tensor(
                    out=w_all[:],
                    in0=gall[:, :, 2:3].rearrange("p k o -> p (k o)"),
                    in1=gall[:, 0, 3:4].to_broadcast([P, K]),
                    op=OP.add)
                nc.scalar.activation(w_all[:], w_all[:], AF.Prelu, alpha=NEG)
                nc.scalar.activation(w_all[:], w_all[:], AF.Exp)
                den = sb.tile([P, 1], f32, tag="den3")
                nc.vector.tensor_reduce(out=den[:], in_=w_all[:],
                                        axis=mybir.AxisListType.X, op=OP.add)
                u_all = sb.tile([P, K, OUT], f32, tag="u_all3")
                nc.vector.tensor_tensor(
                    out=u_all[:],
                    in0=gall[:, :, 0:OUT],
                    in1=w_all[:].to_broadcast([P, K, OUT]),
                    op=OP.mult)
                acc = sb.tile([P, OUT], f32, tag="acc3")
                nc.vector.tensor_reduce(
                    out=acc[:], in_=u_all[:].rearrange("p k o -> p o k"),
                    axis=mybir.AxisListType.X, op=OP.add)
                rden = sb.tile([P, 1], f32, tag="rden3")
                nc.vector.tensor_scalar(out=rden[:], in0=den[:], scalar1=1e-30,
                                        scalar2=None, op0=OP.add)
                nc.vector.reciprocal(rden[:], rden[:])
                o2 = sb.tile([P, OUT], f32, tag="o2")
                nc.vector.tensor_tensor(out=o2[:], in0=acc[:],
                                        in1=rden[:].to_broadcast([P, OUT]),
                                        op=OP.mult)
                nc.vector.tensor_add(o2[:], o2[:], b2_sb[:])
                nc.sync.dma_start(out=out2[t * P:(t + 1) * P, :], in_=o2[:])

            if dump_tables:
                nc.sync.dma_start(out=t1dump[:, :], in_=table1[:])
                nc.sync.dma_start(out=t2dump[:, :], in_=table2[:])

    _split_waits(nc)
    return nc


def _split_waits(nc, max_waits=1):
    """This walrus build allows one sync-wait slot per instruction; hoist
    excess waits onto same-engine InstNoOp carriers inserted just before."""
    import concourse.mybir as mybir
    ctr = 0
    for fn in nc.m.functions:
        for bb in fn.blocks:
            out = []
            changed = False
            for inst in bb.instructions:
                si = inst.sync_info
                if si is not None and len(si.on_wait) > max_waits:
                    waits = list(si.on_wait)
                    extra, keep = waits[:-max_waits], waits[-max_waits:]
                    for i in range(0, len(extra), max_waits):
                        ctr += 1
                        nop = mybir.InstNoOp(name=f"waitfix-{ctr}", ins=[], outs=[])
                        nop.engine = inst.engine
                        nop.sync_info = mybir.SyncInfo(
                            on_wait=extra[i:i + max_waits], on_update=[])
                        out.append(nop)
                    si.on_wait = keep
                    inst.sync_info = si
                    changed = True
                out.append(inst)
            if changed:
                bb.instructions = out
    return ctr


def kernel(x, edge_index, W1, att_src1, att_dst1, b1, W2, att_src2, att_dst2, b2):
    from concourse.bass_utils import run_bass_kernel_spmd

    prep = _host_prep(x, edge_index, W1, att_src1, att_dst1, b1,
                      W2, att_src2, att_dst2, b2)
    nc = _build_program(prep["K_t"], prep["slab_off"], prep["tot_slabs"],
                        prep["K3_tw"], prep["K3_t"], prep["w_off"],
                        prep["colbase"], prep["cols_t"], prep["totcols"])

    in_maps = []
    for c in range(NCORES):
        in_maps.append({
            "x_tiles": prep["x_tiles_pc"][c],
            "w1aug": prep["W1aug_t"],
            "w2aug": prep["W2aug"],
            "dummy1": prep["dummy1"],
            "dummy2": prep["dummy2"],
            "b1_b": prep["b1_b"],
            "b2_b": prep["b2_b"],
            "idx1": np.ascontiguousarray(prep["idx1_pc"][c]),
            "idx2": np.ascontiguousarray(prep["idx2"][c]),
        })

    trace = bool(os.environ.get("GAT_TRACE"))
    if trace:
        _install_ntff_shim()
    r = run_bass_kernel_spmd(nc, in_maps, core_ids=list(range(NCORES)),
                             trace=trace)
    if trace and r.exec_time_ns:
        print(f"HW exec time: {r.exec_time_ns} ns")

    old_of_fid = prep["old_of_fid"]
    out = np.zeros((N, OUT), dtype=np.float32)
    for c in range(NCORES):
        fid0 = c * ROWS_PER_CORE
        olds = old_of_fid[fid0:fid0 + ROWS_PER_CORE]
        m = olds >= 0
        out[olds[m]] = r.results[c]["out2"][m]
    return out


def _install_ntff_shim():
    """The image's antenv lacks axon_hooks; recreate it so trace=True works."""
    import sys, types
    if "antenv.axon_hooks" in sys.modules:
        return
    sys.path.insert(0, "/root/.axon_site/trn_agent_boot")
    try:
        import trn_boot
        hook = trn_boot._ntff_profile_via_ctypes("/opt/axon/libaxon_pjrt.so")
    except Exception:
        hook = None
    mod = types.ModuleType("antenv.axon_hooks")
    mod.get_axon_ntff_profile_hook = lambda: hook
    mod.set_axon_ntff_profile_hook = lambda h: None
    sys.modules["antenv.axon_hooks"] = mod

